# revision 6
# baseline (speedup 1.0000x reference)
"""Trainium2 Bass kernel for grouped-query causal attention (B=2, T=2048, C=1024,
16 q heads / 4 kv heads, RoPE, fused qkv + output projection).

Sharding: 8 cores = (batch b, kv-head h). Each core:
  - projects x -> qT (4 heads), kT, vT with pre-sliced/pre-scaled weights
    (transposed layout: channels on partitions, T on free dim)
  - applies RoPE (pair-swap via permutation matmul on PE + DVE mul/add)
  - causal attention for its 4 query heads (S^T blocks, exp without
    max-subtraction [logits are O(8)], softmax denominators via a ones
    column appended to V, post-exp 0/1 causal mask)
  - partial output projection y^T = Wf_local^T @ oT  (transposed)
Host sums the 4 per-h partials per batch and transposes back.
"""

import sys

sys.path.insert(0, "/opt/trn_rl_repo")

import numpy as np

import concourse.bacc as bacc
import concourse.mybir as mybir
from concourse import tile
from concourse.bass_utils import run_bass_kernel_spmd

B, T, C = 2, 2048, 1024
G, HKV, HS = 4, 4, 64
OUT_DIM = C + 2 * (C // G)
SCALE = 1.0 / np.sqrt(HS)
MAX_PERIOD = 10000.0

F32 = mybir.dt.float32
AF = mybir.ActivationFunctionType

TCH = T // 512  # 4 chunks of 512 along T
NT = T // 128  # 16 tiles of 128 along T


def build_nc():
    nc = bacc.Bacc(None, target_bir_lowering=False)

    xT_d = nc.dram_tensor("xT", [C, T], F32, kind="ExternalInput")
    w_d = nc.dram_tensor("w_qkv", [C, 384], F32, kind="ExternalInput")
    bl_d = nc.dram_tensor("b_loc", [128, 3], F32, kind="ExternalInput")
    cos_d = nc.dram_tensor("cosT", [128, T], F32, kind="ExternalInput")
    sin_d = nc.dram_tensor("sinT", [128, T], F32, kind="ExternalInput")
    perm_d = nc.dram_tensor("perm", [128, 128], F32, kind="ExternalInput")
    eye_d = nc.dram_tensor("eye64", [128, 64], F32, kind="ExternalInput")
    mask_d = nc.dram_tensor("maskb", [128, 896], F32, kind="ExternalInput")
    wf_d = nc.dram_tensor("wf", [256, 1024], F32, kind="ExternalInput")
    bf_d = nc.dram_tensor("bf", [128, 8], F32, kind="ExternalInput")
    yT_d = nc.dram_tensor("yT", [C, T], F32, kind="ExternalOutput")

    with tile.TileContext(nc) as tc:
        with (
            tc.tile_pool(name="persist", bufs=1) as pp,
            tc.tile_pool(name="xstream", bufs=12) as spx,
            tc.tile_pool(name="pstream", bufs=4) as spp,
            tc.tile_pool(name="rstream", bufs=2) as spr,
            tc.tile_pool(name="ostream", bufs=3) as spo,
            tc.tile_pool(name="ps_acc", bufs=2, space="PSUM") as psacc,
            tc.tile_pool(name="ps_tmp", bufs=4, space="PSUM") as ps,
        ):
            # ---- persistent tiles ----
            w_sb = pp.tile([128, 8, 384], F32, tag="w", name="w")
            bl_sb = pp.tile([128, 3], F32, tag="bl", name="bl")
            cos_sb = pp.tile([128, T], F32, tag="cos", name="cos")
            sin_sb = pp.tile([128, T], F32, tag="sin", name="sin")
            perm_sb = pp.tile([128, 128], F32, tag="perm", name="perm")
            eye_sb = pp.tile([128, 64], F32, tag="eye", name="eye")
            mask_sb = pp.tile([128, 896], F32, tag="mask", name="mask")
            wf_sb = pp.tile([128, 2, 1024], F32, tag="wf", name="wf")
            bf_sb = pp.tile([128, 8], F32, tag="bf", name="bf")
            ones_sb = pp.tile([128, 64], F32, tag="ones", name="ones")
            qkvT = [pp.tile([128, T], F32, tag=f"qkvT{m}", name=f"qkvT{m}") for m in range(3)]
            kdup = pp.tile([128, T], F32, tag="kdup", name="kdup")
            v_sb = pp.tile([128, NT, 65], F32, tag="vaug", name="vaug")
            oT_ab = [pp.tile([128, T], F32, tag=f"oT{i}", name=f"oT{i}") for i in range(2)]

            nc.sync.dma_start(w_sb[:], w_d.rearrange("(k p) m -> p k m", p=128))
            nc.sync.dma_start(bl_sb[:], bl_d[:])
            nc.sync.dma_start(cos_sb[:], cos_d[:])
            nc.sync.dma_start(sin_sb[:], sin_d[:])
            nc.sync.dma_start(perm_sb[:], perm_d[:])
            nc.sync.dma_start(eye_sb[:], eye_d[:])
            nc.sync.dma_start(mask_sb[:], mask_d[:])
            nc.sync.dma_start(wf_sb[:], wf_d.rearrange("(c p) n -> p c n", p=128))
            nc.sync.dma_start(bf_sb[:], bf_d[:])
            nc.gpsimd.memset(ones_sb[:], 1.0)
            nc.gpsimd.memset(v_sb[:, :, 64:65], 1.0)

            # ---- phase 1+2: qkv^T projection, bias, RoPE, v transpose ----
            for tci in range(TCH):
                tsl = slice(tci * 512, (tci + 1) * 512)
                xts = []
                for k in range(8):
                    xt = spx.tile([128, 512], F32, tag="xt", name="xt")
                    nc.sync.dma_start(xt[:], xT_d[k * 128 : (k + 1) * 128, tsl])
                    xts.append(xt)
                for mt in range(3):
                    pr = ps.tile([128, 512], F32, tag="tmp", name="tmp")
                    for k in range(8):
                        nc.tensor.matmul(
                            pr[:],
                            w_sb[:, k, mt * 128 : (mt + 1) * 128],
                            xts[k][:],
                            start=(k == 0),
                            stop=(k == 7),
                        )
                    nc.scalar.activation(
                        qkvT[mt][:, tsl], pr[:], AF.Identity, bias=bl_sb[:, mt : mt + 1]
                    )
                # RoPE on q tiles (all 128 partitions = 2 heads each)
                for mt in range(2):
                    tmp = ps.tile([128, 512], F32, tag="tmp", name="tmp")
                    nc.tensor.matmul(
                        tmp[:], perm_sb[:], qkvT[mt][:, tsl], start=True, stop=True
                    )
                    nc.vector.tensor_mul(
                        qkvT[mt][:, tsl], qkvT[mt][:, tsl], cos_sb[:, tsl]
                    )
                    nc.vector.tensor_mul(tmp[:], tmp[:], sin_sb[:, tsl])
                    nc.vector.tensor_add(qkvT[mt][:, tsl], qkvT[mt][:, tsl], tmp[:])
                # RoPE on k rows (partitions 0..64 of tile 2)
                tmp = ps.tile([128, 512], F32, tag="tmp", name="tmp")
                nc.tensor.matmul(
                    tmp[0:64, :],
                    perm_sb[:, 0:64],
                    qkvT[2][:, tsl],
                    start=True,
                    stop=True,
                )
                nc.vector.tensor_mul(
                    qkvT[2][0:64, tsl], qkvT[2][0:64, tsl], cos_sb[0:64, tsl]
                )
                nc.vector.tensor_mul(tmp[0:64, :], tmp[0:64, :], sin_sb[0:64, tsl])
                nc.vector.tensor_add(
                    qkvT[2][0:64, tsl], qkvT[2][0:64, tsl], tmp[0:64, :]
                )
                # duplicate rotated k at partition base 64 for odd q-heads
                nc.sync.dma_start(kdup[64:128, tsl], qkvT[2][0:64, tsl])
                # v transpose: (64, 128) blocks -> (128, 64) row-major v
                for i in range(4):
                    tt = tci * 4 + i
                    vt = ps.tile([128, 512], F32, tag="tmp", name="tmp")
                    nc.tensor.transpose(
                        vt[:, 0:64],
                        qkvT[2][64:128, tt * 128 : (tt + 1) * 128],
                        eye_sb[64:128, :],
                    )
                    nc.vector.tensor_copy(v_sb[:, tt, 0:64], vt[:, 0:64])

            # ---- phase 3: attention per (head, tq-chunk) ----
            for g in range(G):
                qtile = qkvT[g // 2]
                qrow = (g % 2) * 64
                odd = g % 2 == 1
                for tci in range(TCH):
                    tsl = slice(tci * 512, (tci + 1) * 512)
                    nblk = 4 * tci + 4
                    o_ac = psacc.tile([128, 512], F32, tag="oacc", name="oacc")
                    for j in range(nblk):
                        s_ps = ps.tile([128, 512], F32, tag="tmp", name="tmp")
                        ksrc = (
                            kdup[64:128, j * 128 : (j + 1) * 128]
                            if odd
                            else qkvT[2][0:64, j * 128 : (j + 1) * 128]
                        )
                        nc.tensor.matmul(
                            s_ps[:],
                            ksrc,
                            qtile[qrow : qrow + 64, tsl],
                            start=True,
                            stop=True,
                        )
                        p_sb = spp.tile([128, 512], F32, tag="p", name="p")
                        nc.scalar.activation(p_sb[:], s_ps[:], AF.Exp)
                        if j >= 4 * tci:
                            off = 384 + 512 * tci - 128 * j
                            nc.gpsimd.tensor_mul(
                                p_sb[:], p_sb[:], mask_sb[:, off : off + 512]
                            )
                        nc.tensor.matmul(
                            o_ac[0:65, :],
                            v_sb[:, j, 0:65],
                            p_sb[:],
                            start=(j == 0),
                            stop=(j == nblk - 1),
                        )
                    rec = spr.tile([128, 512], F32, tag="rec", name="rec")
                    nc.vector.reciprocal(rec[64:65, :], o_ac[64:65, :])
                    bc = ps.tile([128, 512], F32, tag="tmp", name="tmp")
                    nc.tensor.matmul(
                        bc[0:64, :],
                        ones_sb[64:65, 0:64],
                        rec[64:65, :],
                        start=True,
                        stop=True,
                    )
                    bc_sb = spr.tile([64, 512], F32, tag="bcs", name="bcs")
                    nc.vector.tensor_copy(bc_sb[:], bc[0:64, :])
                    if odd:
                        stg = spr.tile([64, 512], F32, tag="stg", name="stg")
                        nc.vector.tensor_mul(stg[:], o_ac[0:64, :], bc_sb[:])
                        nc.sync.dma_start(oT_ab[g // 2][64:128, tsl], stg[:])
                    else:
                        nc.vector.tensor_mul(
                            oT_ab[g // 2][0:64, tsl], o_ac[0:64, :], bc_sb[:]
                        )

            # ---- phase 4: partial final projection y^T = wf^T @ oT + bias ----
            for tci in range(TCH):
                tsl = slice(tci * 512, (tci + 1) * 512)
                for nt in range(8):
                    y_ps = ps.tile([128, 512], F32, tag="tmp", name="tmp")
                    for cc in range(2):
                        nc.tensor.matmul(
                            y_ps[:],
                            wf_sb[:, cc, nt * 128 : (nt + 1) * 128],
                            oT_ab[cc][:, tsl],
                            start=(cc == 0),
                            stop=(cc == 1),
                        )
                    y_sb = spo.tile([128, 512], F32, tag="yout", name="yout")
                    nc.scalar.activation(
                        y_sb[:], y_ps[:], AF.Identity, bias=bf_sb[:, nt : nt + 1]
                    )
                    nc.sync.dma_start(yT_d[nt * 128 : (nt + 1) * 128, tsl], y_sb[:])

    nc.compile()
    return nc


def host_shard(inputs):
    """Build the 8 per-core input maps from full inputs."""
    x = np.ascontiguousarray(np.asarray(inputs["input"], dtype=np.float32))
    W = np.asarray(inputs["W_attn"], dtype=np.float32)
    bb = np.asarray(inputs["b_attn"], dtype=np.float32)
    Wf = np.asarray(inputs["W_final"], dtype=np.float32)
    bf = np.asarray(inputs["b_final"], dtype=np.float32)

    half = HS // 2
    inv_freq = MAX_PERIOD ** (-np.arange(half, dtype=np.float32) / half)
    ang = np.arange(T, dtype=np.float32)[:, None] * inv_freq  # (T, 32)
    sin_t = np.sin(ang).astype(np.float32)
    cos_t = np.cos(ang).astype(np.float32)
    cosT = np.repeat(cos_t.T, 2, axis=0)  # (64, T): row d -> cos(t*f[d//2])
    sgn = np.where(np.arange(HS) % 2 == 0, -1.0, 1.0).astype(np.float32)
    sinT = np.repeat(sin_t.T, 2, axis=0) * sgn[:, None]
    cos128 = np.ascontiguousarray(np.concatenate([cosT, cosT], axis=0))
    sin128 = np.ascontiguousarray(np.concatenate([sinT, sinT], axis=0))

    perm = np.zeros((128, 128), np.float32)
    idx = np.arange(128)
    perm[idx ^ 1, idx] = 1.0
    eye64 = np.zeros((128, 64), np.float32)
    eye64[64:128, :] = np.eye(64, dtype=np.float32)
    u = np.arange(896)
    maskb = (u[None, :] >= (np.arange(128)[:, None] + 384)).astype(np.float32)

    in_maps = []
    for cid in range(8):
        b, h = cid // 4, cid % 4
        qcols = np.concatenate(
            [np.arange(g * 256 + h * 64, g * 256 + h * 64 + 64) for g in range(G)]
        )
        kcols = np.arange(1024 + h * 64, 1024 + h * 64 + 64)
        vcols = np.arange(1280 + h * 64, 1280 + h * 64 + 64)
        cols = np.concatenate([qcols, kcols, vcols])
        w_loc = W[:, cols].copy()
        b_loc = bb[cols].copy()
        w_loc[:, :256] *= SCALE
        b_loc[:256] *= SCALE
        b_loc_m = np.ascontiguousarray(b_loc.reshape(3, 128).T)  # (128, 3)

        rows = np.concatenate(
            [np.arange(g * 256 + h * 64, g * 256 + h * 64 + 64) for g in range(G)]
        )
        wf_loc = np.ascontiguousarray(Wf[rows, :])  # (256, 1024)
        bf_m = (
            np.ascontiguousarray(bf.reshape(8, 128).T)
            if h == 0
            else np.zeros((128, 8), np.float32)
        )

        in_maps.append(
            {
                "xT": np.ascontiguousarray(x[b].T),
                "w_qkv": w_loc,
                "b_loc": b_loc_m,
                "cosT": cos128,
                "sinT": sin128,
                "perm": perm,
                "eye64": eye64,
                "maskb": maskb,
                "wf": wf_loc,
                "bf": bf_m,
            }
        )
    return in_maps


def host_unshard(results):
    """Sum the 4 per-h partial yT per batch, transpose back to (B, T, C)."""
    out = np.empty((B, T, C), np.float32)
    for b in range(B):
        acc = results[b * 4]["yT"].astype(np.float32)
        for h in range(1, 4):
            acc = acc + results[b * 4 + h]["yT"]
        out[b] = acc.T
    return out


_NC_CACHE = None


def _get_nc():
    global _NC_CACHE
    if _NC_CACHE is None:
        _NC_CACHE = build_nc()
    return _NC_CACHE


def kernel(**inputs):
    nc = _get_nc()
    in_maps = host_shard(inputs)
    res = run_bass_kernel_spmd(nc, in_maps, core_ids=list(range(8)))
    return host_unshard(res.results)


# revision 12
# speedup vs baseline: 1.7621x; 1.7621x over previous
"""Trainium2 Bass kernel for grouped-query causal attention (B=2, T=2048, C=1024,
16 q heads / 4 kv heads, RoPE, fused qkv + output projection).

Sharding: 8 cores = (batch b, kv-head h). Each core:
  - projects x -> qT (4 heads), kT, vT with pre-sliced/pre-scaled weights
    (transposed layout: channels on partitions, T on free dim)
  - applies RoPE (pair-swap via permutation matmul on PE + DVE mul/add)
  - causal attention for its 4 query heads (S^T blocks, exp without
    max-subtraction [logits are O(8)], softmax denominators via a ones
    column appended to V, post-exp 0/1 causal mask)
  - partial output projection y^T = Wf_local^T @ oT  (transposed)
Host sums the 4 per-h partials per batch and transposes back.
"""

import sys

sys.path.insert(0, "/opt/trn_rl_repo")

import numpy as np

import concourse.bacc as bacc
import concourse.mybir as mybir
from concourse import tile
from concourse.bass_utils import run_bass_kernel_spmd

B, T, C = 2, 2048, 1024
G, HKV, HS = 4, 4, 64
OUT_DIM = C + 2 * (C // G)
SCALE = 1.0 / np.sqrt(HS)
MAX_PERIOD = 10000.0

F32 = mybir.dt.float32
F32R = mybir.dt.float32r
AF = mybir.ActivationFunctionType


TCH = T // 512  # 4 chunks of 512 along T
NT = T // 128  # 16 tiles of 128 along T


def build_nc():
    nc = bacc.Bacc(None, target_bir_lowering=False)

    xT_d = nc.dram_tensor("xT", [C, T], F32R, kind="ExternalInput")
    w_d = nc.dram_tensor("w_qkv", [C, 384], F32R, kind="ExternalInput")
    bl_d = nc.dram_tensor("b_loc", [128, 3], F32, kind="ExternalInput")
    cos_d = nc.dram_tensor("cosT", [128, T], F32R, kind="ExternalInput")
    sin_d = nc.dram_tensor("sinT", [128, T], F32R, kind="ExternalInput")
    perm_d = nc.dram_tensor("perm", [128, 128], F32R, kind="ExternalInput")
    eye_d = nc.dram_tensor("eye64", [128, 64], F32R, kind="ExternalInput")
    mask_d = nc.dram_tensor("maskb", [128, 896], F32R, kind="ExternalInput")
    wf_d = nc.dram_tensor("wf", [256, 1024], F32R, kind="ExternalInput")
    bf_d = nc.dram_tensor("bf", [128, 8], F32, kind="ExternalInput")
    ones_d = nc.dram_tensor("onesd", [128, 64], F32R, kind="ExternalInput")
    yT_d = nc.dram_tensor("yT", [C, T], F32, kind="ExternalOutput")

    with tile.TileContext(nc) as tc:
        with (
            tc.tile_pool(name="persist", bufs=1) as pp,
            tc.tile_pool(name="xstream", bufs=12) as spx,
            tc.tile_pool(name="pstream", bufs=4) as spp,
            tc.tile_pool(name="rstream", bufs=2) as spr,
            tc.tile_pool(name="ostream", bufs=3) as spo,
            tc.tile_pool(name="ps_acc", bufs=2, space="PSUM") as psacc,
            tc.tile_pool(name="ps_tmp", bufs=4, space="PSUM") as ps,
        ):
            # ---- persistent tiles ----
            w_sb = pp.tile([128, 8, 384], F32R, tag="w", name="w")
            bl_sb = pp.tile([128, 3], F32, tag="bl", name="bl")
            cos_sb = pp.tile([128, T], F32R, tag="cos", name="cos")
            sin_sb = pp.tile([128, T], F32R, tag="sin", name="sin")
            perm_sb = pp.tile([128, 128], F32R, tag="perm", name="perm")
            eye_sb = pp.tile([128, 64], F32R, tag="eye", name="eye")
            mask_sb = pp.tile([128, 896], F32R, tag="mask", name="mask")
            wf_sb = pp.tile([128, 2, 1024], F32R, tag="wf", name="wf")
            bf_sb = pp.tile([128, 8], F32, tag="bf", name="bf")
            ones_sb = pp.tile([128, 64], F32R, tag="ones", name="ones")
            qkvT = [pp.tile([128, T], F32R, tag=f"qkvT{m}", name=f"qkvT{m}") for m in range(3)]
            kdup = pp.tile([128, T], F32R, tag="kdup", name="kdup")
            v_sb = pp.tile([128, NT, 65], F32R, tag="vaug", name="vaug")
            oT_ab = [pp.tile([128, T], F32R, tag=f"oT{i}", name=f"oT{i}") for i in range(2)]

            nc.sync.dma_start(w_sb[:], w_d.rearrange("(k p) m -> p k m", p=128))
            nc.sync.dma_start(bl_sb[:], bl_d[:])
            nc.sync.dma_start(cos_sb[:], cos_d[:])
            nc.sync.dma_start(sin_sb[:], sin_d[:])
            nc.sync.dma_start(perm_sb[:], perm_d[:])
            nc.sync.dma_start(eye_sb[:], eye_d[:])
            nc.sync.dma_start(mask_sb[:], mask_d[:])
            nc.sync.dma_start(wf_sb[:], wf_d.rearrange("(c p) n -> p c n", p=128))
            nc.sync.dma_start(bf_sb[:], bf_d[:])
            nc.sync.dma_start(ones_sb[:], ones_d[:])
            nc.sync.dma_start(v_sb[:, :, 64:65], ones_d[:, 0:NT])

            # ---- phase 1+2: qkv^T projection, bias, RoPE, v transpose ----
            for tci in range(TCH):
                tsl = slice(tci * 512, (tci + 1) * 512)
                xts = []
                for k in range(8):
                    xt = spx.tile([128, 512], F32R, tag="xt", name="xt")
                    nc.sync.dma_start(xt[:], xT_d[k * 128 : (k + 1) * 128, tsl])
                    xts.append(xt)
                for mt in range(3):
                    pr = ps.tile([128, 512], F32, tag="tmp", name="tmp")
                    for k in range(8):
                        nc.tensor.matmul(
                            pr[:],
                            (w_sb[:, k, mt * 128 : (mt + 1) * 128]),
                            (xts[k][:]),
                            start=(k == 0),
                            stop=(k == 7),
                        )
                    nc.scalar.activation(
                        qkvT[mt][:, tsl], pr[:], AF.Identity, bias=bl_sb[:, mt : mt + 1]
                    )
                # RoPE on q tiles (all 128 partitions = 2 heads each)
                for mt in range(2):
                    tmp = ps.tile([128, 512], F32, tag="tmp", name="tmp")
                    nc.tensor.matmul(
                        tmp[:], (perm_sb[:]), (qkvT[mt][:, tsl]), start=True, stop=True
                    )
                    nc.vector.tensor_mul(
                        qkvT[mt][:, tsl], qkvT[mt][:, tsl], cos_sb[:, tsl]
                    )
                    nc.vector.tensor_mul(tmp[:], tmp[:], sin_sb[:, tsl])
                    nc.vector.tensor_add(qkvT[mt][:, tsl], qkvT[mt][:, tsl], tmp[:])
                # RoPE on k rows (partitions 0..64 of tile 2)
                tmp = ps.tile([128, 512], F32, tag="tmp", name="tmp")
                nc.tensor.matmul(
                    tmp[0:64, :],
                    (perm_sb[:, 0:64]),
                    (qkvT[2][:, tsl]),
                    start=True,
                    stop=True,
                )
                nc.vector.tensor_mul(
                    qkvT[2][0:64, tsl], qkvT[2][0:64, tsl], cos_sb[0:64, tsl]
                )
                nc.vector.tensor_mul(tmp[0:64, :], tmp[0:64, :], sin_sb[0:64, tsl])
                nc.vector.tensor_add(
                    qkvT[2][0:64, tsl], qkvT[2][0:64, tsl], tmp[0:64, :]
                )
                # duplicate rotated k at partition base 64 for odd q-heads
                nc.sync.dma_start(kdup[64:128, tsl], qkvT[2][0:64, tsl])
                # v transpose: (64, 128) blocks -> (128, 64) row-major v
                for i in range(4):
                    tt = tci * 4 + i
                    vt = ps.tile([128, 512], F32, tag="tmp", name="tmp")
                    nc.tensor.transpose(
                        vt[:, 0:64].bitcast(F32R),
                        (qkvT[2][64:128, tt * 128 : (tt + 1) * 128]),
                        (eye_sb[64:128, :]),
                    )
                    nc.vector.tensor_copy(v_sb[:, tt, 0:64], vt[:, 0:64])

            # ---- phase 3: attention per (head, tq-chunk) ----
            for g in range(G):
                qtile = qkvT[g // 2]
                qrow = (g % 2) * 64
                odd = g % 2 == 1
                for tci in range(TCH):
                    tsl = slice(tci * 512, (tci + 1) * 512)
                    nblk = 4 * tci + 4
                    o_ac = psacc.tile([128, 512], F32, tag="oacc", name="oacc")
                    for j in range(nblk):
                        s_ps = ps.tile([128, 512], F32, tag="tmp", name="tmp")
                        ksrc = (
                            kdup[64:128, j * 128 : (j + 1) * 128]
                            if odd
                            else qkvT[2][0:64, j * 128 : (j + 1) * 128]
                        )
                        nc.tensor.matmul(
                            s_ps[:],
                            (ksrc),
                            (qtile[qrow : qrow + 64, tsl]),
                            start=True,
                            stop=True,
                        )
                        p_sb = spp.tile([128, 512], F32R, tag="p", name="p")
                        nc.scalar.activation(p_sb[:], s_ps[:], AF.Exp)
                        if j >= 4 * tci:
                            off = 384 + 512 * tci - 128 * j
                            nc.gpsimd.tensor_mul(
                                p_sb[:], p_sb[:], mask_sb[:, off : off + 512]
                            )
                        nc.tensor.matmul(
                            o_ac[0:65, :],
                            (v_sb[:, j, 0:65]),
                            (p_sb[:]),
                            start=(j == 0),
                            stop=(j == nblk - 1),
                        )
                    rec = spr.tile([128, 512], F32R, tag="rec", name="rec")
                    with nc.allow_low_precision(reason="f32r softmax denom"):
                        nc.vector.reciprocal(rec[64:65, :], o_ac[64:65, :])
                    bc = ps.tile([128, 512], F32, tag="tmp", name="tmp")
                    nc.tensor.matmul(
                        bc[0:64, :],
                        (ones_sb[64:65, 0:64]),
                        (rec[64:65, :]),
                        start=True,
                        stop=True,
                    )
                    bc_sb = spr.tile([64, 512], F32, tag="bcs", name="bcs")
                    nc.vector.tensor_copy(bc_sb[:], bc[0:64, :])
                    if odd:
                        stg = spr.tile([64, 512], F32R, tag="stg", name="stg")
                        nc.vector.tensor_mul(stg[:], o_ac[0:64, :], bc_sb[:])
                        nc.sync.dma_start(oT_ab[g // 2][64:128, tsl], stg[:])
                    else:
                        nc.vector.tensor_mul(
                            oT_ab[g // 2][0:64, tsl], o_ac[0:64, :], bc_sb[:]
                        )

            # ---- phase 4: partial final projection y^T = wf^T @ oT + bias ----
            for tci in range(TCH):
                tsl = slice(tci * 512, (tci + 1) * 512)
                for nt in range(8):
                    y_ps = ps.tile([128, 512], F32, tag="tmp", name="tmp")
                    for cc in range(2):
                        nc.tensor.matmul(
                            y_ps[:],
                            (wf_sb[:, cc, nt * 128 : (nt + 1) * 128]),
                            (oT_ab[cc][:, tsl]),
                            start=(cc == 0),
                            stop=(cc == 1),
                        )
                    y_sb = spo.tile([128, 512], F32, tag="yout", name="yout")
                    nc.scalar.activation(
                        y_sb[:], y_ps[:], AF.Identity, bias=bf_sb[:, nt : nt + 1]
                    )
                    nc.sync.dma_start(yT_d[nt * 128 : (nt + 1) * 128, tsl], y_sb[:])

    nc.compile()
    return nc


def host_shard(inputs):
    """Build the 8 per-core input maps from full inputs."""
    x = np.ascontiguousarray(np.asarray(inputs["input"], dtype=np.float32))
    W = np.asarray(inputs["W_attn"], dtype=np.float32)
    bb = np.asarray(inputs["b_attn"], dtype=np.float32)
    Wf = np.asarray(inputs["W_final"], dtype=np.float32)
    bf = np.asarray(inputs["b_final"], dtype=np.float32)

    half = HS // 2
    inv_freq = MAX_PERIOD ** (-np.arange(half, dtype=np.float32) / half)
    ang = np.arange(T, dtype=np.float32)[:, None] * inv_freq  # (T, 32)
    sin_t = np.sin(ang).astype(np.float32)
    cos_t = np.cos(ang).astype(np.float32)
    cosT = np.repeat(cos_t.T, 2, axis=0)  # (64, T): row d -> cos(t*f[d//2])
    sgn = np.where(np.arange(HS) % 2 == 0, -1.0, 1.0).astype(np.float32)
    sinT = np.repeat(sin_t.T, 2, axis=0) * sgn[:, None]
    cos128 = np.ascontiguousarray(np.concatenate([cosT, cosT], axis=0))
    sin128 = np.ascontiguousarray(np.concatenate([sinT, sinT], axis=0))

    perm = np.zeros((128, 128), np.float32)
    idx = np.arange(128)
    perm[idx ^ 1, idx] = 1.0
    eye64 = np.zeros((128, 64), np.float32)
    eye64[64:128, :] = np.eye(64, dtype=np.float32)
    u = np.arange(896)
    maskb = (u[None, :] >= (np.arange(128)[:, None] + 384)).astype(np.float32)

    in_maps = []
    for cid in range(8):
        b, h = cid // 4, cid % 4
        qcols = np.concatenate(
            [np.arange(g * 256 + h * 64, g * 256 + h * 64 + 64) for g in range(G)]
        )
        kcols = np.arange(1024 + h * 64, 1024 + h * 64 + 64)
        vcols = np.arange(1280 + h * 64, 1280 + h * 64 + 64)
        cols = np.concatenate([qcols, kcols, vcols])
        w_loc = W[:, cols].copy()
        b_loc = bb[cols].copy()
        w_loc[:, :256] *= SCALE
        b_loc[:256] *= SCALE
        b_loc_m = np.ascontiguousarray(b_loc.reshape(3, 128).T)  # (128, 3)

        rows = np.concatenate(
            [np.arange(g * 256 + h * 64, g * 256 + h * 64 + 64) for g in range(G)]
        )
        wf_loc = np.ascontiguousarray(Wf[rows, :])  # (256, 1024)
        bf_m = (
            np.ascontiguousarray(bf.reshape(8, 128).T)
            if h == 0
            else np.zeros((128, 8), np.float32)
        )

        in_maps.append(
            {
                "xT": np.ascontiguousarray(x[b].T),
                "w_qkv": w_loc,
                "b_loc": b_loc_m,
                "cosT": cos128,
                "sinT": sin128,
                "perm": perm,
                "eye64": eye64,
                "maskb": maskb,
                "wf": wf_loc,
                "bf": bf_m,
                "onesd": np.ones((128, 64), np.float32),
            }
        )
    return in_maps


def host_unshard(results):
    """Sum the 4 per-h partial yT per batch, transpose back to (B, T, C)."""
    out = np.empty((B, T, C), np.float32)
    for b in range(B):
        acc = results[b * 4]["yT"].astype(np.float32)
        for h in range(1, 4):
            acc = acc + results[b * 4 + h]["yT"]
        out[b] = acc.T
    return out


_NC_CACHE = None


def _get_nc():
    global _NC_CACHE
    if _NC_CACHE is None:
        _NC_CACHE = build_nc()
    return _NC_CACHE


def kernel(**inputs):
    nc = _get_nc()
    in_maps = host_shard(inputs)
    res = run_bass_kernel_spmd(nc, in_maps, core_ids=list(range(8)))
    return host_unshard(res.results)


# revision 13
# speedup vs baseline: 1.9597x; 1.1122x over previous
"""Trainium2 Bass kernel for grouped-query causal attention (B=2, T=2048, C=1024,
16 q heads / 4 kv heads, RoPE, fused qkv + output projection).

Sharding: 8 cores = (batch b, kv-head h). Each core:
  - projects x -> qT (4 heads), kT, vT with pre-sliced/pre-scaled weights
    (transposed layout: channels on partitions, T on free dim)
  - applies RoPE (pair-swap via permutation matmul on PE + DVE mul/add)
  - causal attention for its 4 query heads (S^T blocks, exp without
    max-subtraction [logits are O(8)], softmax denominators via a ones
    column appended to V, post-exp 0/1 causal mask)
  - partial output projection y^T = Wf_local^T @ oT  (transposed)
Host sums the 4 per-h partials per batch and transposes back.
"""

import sys

sys.path.insert(0, "/opt/trn_rl_repo")

import numpy as np

import concourse.bacc as bacc
import concourse.mybir as mybir
from concourse import tile
from concourse.bass_utils import run_bass_kernel_spmd

B, T, C = 2, 2048, 1024
G, HKV, HS = 4, 4, 64
OUT_DIM = C + 2 * (C // G)
SCALE = 1.0 / np.sqrt(HS)
MAX_PERIOD = 10000.0

F32 = mybir.dt.float32
F32R = mybir.dt.float32r
AF = mybir.ActivationFunctionType


TCH = T // 512  # 4 chunks of 512 along T
NT = T // 128  # 16 tiles of 128 along T


def build_nc():
    nc = bacc.Bacc(None, target_bir_lowering=False)

    xT_d = nc.dram_tensor("xT", [C, T], F32R, kind="ExternalInput")
    w_d = nc.dram_tensor("w_qkv", [C, 384], F32R, kind="ExternalInput")
    bl_d = nc.dram_tensor("b_loc", [128, 3], F32, kind="ExternalInput")
    cos_d = nc.dram_tensor("cosT", [128, T], F32R, kind="ExternalInput")
    sin_d = nc.dram_tensor("sinT", [128, T], F32R, kind="ExternalInput")
    perm_d = nc.dram_tensor("perm", [128, 128], F32R, kind="ExternalInput")
    eye_d = nc.dram_tensor("eye64", [128, 64], F32R, kind="ExternalInput")
    mask_d = nc.dram_tensor("maskb", [128, 896], F32R, kind="ExternalInput")
    wf_d = nc.dram_tensor("wf", [256, 1024], F32R, kind="ExternalInput")
    bf_d = nc.dram_tensor("bf", [128, 8], F32, kind="ExternalInput")
    ones_d = nc.dram_tensor("onesd", [128, 64], F32R, kind="ExternalInput")
    yT_d = nc.dram_tensor("yT", [C, T], F32, kind="ExternalOutput")

    with tile.TileContext(nc) as tc:
        with (
            tc.tile_pool(name="persist", bufs=1) as pp,
            tc.tile_pool(name="xstream", bufs=12) as spx,
            tc.tile_pool(name="pstream", bufs=6) as spp,
            tc.tile_pool(name="rstream", bufs=3) as spr,
            tc.tile_pool(name="ostream", bufs=3) as spo,
            tc.tile_pool(name="ps_acc", bufs=3, space="PSUM") as psacc,
            tc.tile_pool(name="ps_tmp", bufs=4, space="PSUM") as ps,
        ):
            # ---- persistent tiles ----
            w_sb = pp.tile([128, 8, 384], F32R, tag="w", name="w")
            bl_sb = pp.tile([128, 3], F32, tag="bl", name="bl")
            cos_sb = pp.tile([128, T], F32R, tag="cos", name="cos")
            sin_sb = pp.tile([128, T], F32R, tag="sin", name="sin")
            perm_sb = pp.tile([128, 128], F32R, tag="perm", name="perm")
            eye_sb = pp.tile([128, 64], F32R, tag="eye", name="eye")
            mask_sb = pp.tile([128, 896], F32R, tag="mask", name="mask")
            wf_sb = pp.tile([128, 2, 1024], F32R, tag="wf", name="wf")
            bf_sb = pp.tile([128, 8], F32, tag="bf", name="bf")
            ones_sb = pp.tile([128, 64], F32R, tag="ones", name="ones")
            qkvT = [pp.tile([128, T], F32R, tag=f"qkvT{m}", name=f"qkvT{m}") for m in range(3)]
            kdup = pp.tile([128, T], F32R, tag="kdup", name="kdup")
            v_sb = pp.tile([128, NT, 65], F32R, tag="vaug", name="vaug")
            oT_ab = [pp.tile([128, T], F32R, tag=f"oT{i}", name=f"oT{i}") for i in range(2)]

            nc.sync.dma_start(w_sb[:], w_d.rearrange("(k p) m -> p k m", p=128))
            nc.sync.dma_start(bl_sb[:], bl_d[:])
            nc.sync.dma_start(cos_sb[:], cos_d[:])
            nc.sync.dma_start(sin_sb[:], sin_d[:])
            nc.sync.dma_start(perm_sb[:], perm_d[:])
            nc.sync.dma_start(eye_sb[:], eye_d[:])
            nc.sync.dma_start(mask_sb[:], mask_d[:])
            nc.sync.dma_start(wf_sb[:], wf_d.rearrange("(c p) n -> p c n", p=128))
            nc.sync.dma_start(bf_sb[:], bf_d[:])
            nc.sync.dma_start(ones_sb[:], ones_d[:])
            nc.sync.dma_start(v_sb[:, :, 64:65], ones_d[:, 0:NT])

            # ---- phase 1+2: qkv^T projection, bias, RoPE, v transpose ----
            for tci in range(TCH):
                tsl = slice(tci * 512, (tci + 1) * 512)
                xts = []
                for k in range(8):
                    xt = spx.tile([128, 512], F32R, tag="xt", name="xt")
                    nc.sync.dma_start(xt[:], xT_d[k * 128 : (k + 1) * 128, tsl])
                    xts.append(xt)
                for mt in range(3):
                    pr = ps.tile([128, 512], F32, tag="tmp", name="tmp")
                    for k in range(8):
                        nc.tensor.matmul(
                            pr[:],
                            (w_sb[:, k, mt * 128 : (mt + 1) * 128]),
                            (xts[k][:]),
                            start=(k == 0),
                            stop=(k == 7),
                        )
                    nc.vector.tensor_scalar_add(
                        qkvT[mt][:, tsl], pr[:], bl_sb[:, mt : mt + 1]
                    )
                # RoPE on q tiles (all 128 partitions = 2 heads each)
                for mt in range(2):
                    tmp = ps.tile([128, 512], F32, tag="tmp", name="tmp")
                    nc.tensor.matmul(
                        tmp[:], (perm_sb[:]), (qkvT[mt][:, tsl]), start=True, stop=True
                    )
                    nc.vector.tensor_mul(
                        qkvT[mt][:, tsl], qkvT[mt][:, tsl], cos_sb[:, tsl]
                    )
                    nc.vector.tensor_mul(tmp[:], tmp[:], sin_sb[:, tsl])
                    nc.vector.tensor_add(qkvT[mt][:, tsl], qkvT[mt][:, tsl], tmp[:])
                # RoPE on k rows (partitions 0..64 of tile 2)
                tmp = ps.tile([128, 512], F32, tag="tmp", name="tmp")
                nc.tensor.matmul(
                    tmp[0:64, :],
                    (perm_sb[:, 0:64]),
                    (qkvT[2][:, tsl]),
                    start=True,
                    stop=True,
                )
                nc.vector.tensor_mul(
                    qkvT[2][0:64, tsl], qkvT[2][0:64, tsl], cos_sb[0:64, tsl]
                )
                nc.vector.tensor_mul(tmp[0:64, :], tmp[0:64, :], sin_sb[0:64, tsl])
                nc.vector.tensor_add(
                    qkvT[2][0:64, tsl], qkvT[2][0:64, tsl], tmp[0:64, :]
                )
                # duplicate rotated k at partition base 64 for odd q-heads
                nc.sync.dma_start(kdup[64:128, tsl], qkvT[2][0:64, tsl])
                # v transpose: (64, 128) blocks -> (128, 64) row-major v
                for i in range(4):
                    tt = tci * 4 + i
                    vt = ps.tile([128, 512], F32, tag="tmp", name="tmp")
                    nc.tensor.transpose(
                        vt[:, 0:64].bitcast(F32R),
                        (qkvT[2][64:128, tt * 128 : (tt + 1) * 128]),
                        (eye_sb[64:128, :]),
                    )
                    nc.vector.tensor_copy(v_sb[:, tt, 0:64], vt[:, 0:64])

            # ---- phase 3: attention per (head, tq-chunk) ----
            for g in range(G):
                qtile = qkvT[g // 2]
                qrow = (g % 2) * 64
                odd = g % 2 == 1
                for tci in range(TCH):
                    tsl = slice(tci * 512, (tci + 1) * 512)
                    nblk = 4 * tci + 4
                    o_ac = psacc.tile([128, 512], F32, tag="oacc", name="oacc")
                    for j in range(nblk):
                        s_ps = ps.tile([128, 512], F32, tag="tmp", name="tmp")
                        ksrc = (
                            kdup[64:128, j * 128 : (j + 1) * 128]
                            if odd
                            else qkvT[2][0:64, j * 128 : (j + 1) * 128]
                        )
                        nc.tensor.matmul(
                            s_ps[:],
                            (ksrc),
                            (qtile[qrow : qrow + 64, tsl]),
                            start=True,
                            stop=True,
                        )
                        p_sb = spp.tile([128, 512], F32R, tag="p", name="p")
                        nc.scalar.activation(p_sb[:], s_ps[:], AF.Exp)
                        if j >= 4 * tci:
                            off = 384 + 512 * tci - 128 * j
                            nc.gpsimd.tensor_mul(
                                p_sb[:], p_sb[:], mask_sb[:, off : off + 512]
                            )
                        nc.tensor.matmul(
                            o_ac[0:65, :],
                            (v_sb[:, j, 0:65]),
                            (p_sb[:]),
                            start=(j == 0),
                            stop=(j == nblk - 1),
                        )
                    sums = spr.tile([128, 512], F32R, tag="rec", name="rec")
                    nc.vector.tensor_copy(sums[64:65, :], o_ac[64:65, :])
                    bc = ps.tile([128, 512], F32, tag="tmp", name="tmp")
                    nc.tensor.matmul(
                        bc[0:64, :],
                        (ones_sb[64:65, 0:64]),
                        (sums[64:65, :]),
                        start=True,
                        stop=True,
                    )
                    bc_sb = spr.tile([64, 512], F32, tag="bcs", name="bcs")
                    nc.vector.reciprocal_approx_fast(out=bc_sb[:], in_=bc[0:64, :])
                    if odd:
                        stg = spr.tile([64, 512], F32R, tag="stg", name="stg")
                        nc.vector.tensor_mul(stg[:], o_ac[0:64, :], bc_sb[:])
                        nc.sync.dma_start(oT_ab[g // 2][64:128, tsl], stg[:])
                    else:
                        nc.vector.tensor_mul(
                            oT_ab[g // 2][0:64, tsl], o_ac[0:64, :], bc_sb[:]
                        )

            # ---- phase 4: partial final projection y^T = wf^T @ oT + bias ----
            for tci in range(TCH):
                tsl = slice(tci * 512, (tci + 1) * 512)
                for nt in range(8):
                    y_ps = ps.tile([128, 512], F32, tag="tmp", name="tmp")
                    for cc in range(2):
                        nc.tensor.matmul(
                            y_ps[:],
                            (wf_sb[:, cc, nt * 128 : (nt + 1) * 128]),
                            (oT_ab[cc][:, tsl]),
                            start=(cc == 0),
                            stop=(cc == 1),
                        )
                    y_sb = spo.tile([128, 512], F32, tag="yout", name="yout")
                    nc.vector.tensor_scalar_add(
                        y_sb[:], y_ps[:], bf_sb[:, nt : nt + 1]
                    )
                    nc.sync.dma_start(yT_d[nt * 128 : (nt + 1) * 128, tsl], y_sb[:])

    nc.compile()
    return nc


def host_shard(inputs):
    """Build the 8 per-core input maps from full inputs."""
    x = np.ascontiguousarray(np.asarray(inputs["input"], dtype=np.float32))
    W = np.asarray(inputs["W_attn"], dtype=np.float32)
    bb = np.asarray(inputs["b_attn"], dtype=np.float32)
    Wf = np.asarray(inputs["W_final"], dtype=np.float32)
    bf = np.asarray(inputs["b_final"], dtype=np.float32)

    half = HS // 2
    inv_freq = MAX_PERIOD ** (-np.arange(half, dtype=np.float32) / half)
    ang = np.arange(T, dtype=np.float32)[:, None] * inv_freq  # (T, 32)
    sin_t = np.sin(ang).astype(np.float32)
    cos_t = np.cos(ang).astype(np.float32)
    cosT = np.repeat(cos_t.T, 2, axis=0)  # (64, T): row d -> cos(t*f[d//2])
    sgn = np.where(np.arange(HS) % 2 == 0, -1.0, 1.0).astype(np.float32)
    sinT = np.repeat(sin_t.T, 2, axis=0) * sgn[:, None]
    cos128 = np.ascontiguousarray(np.concatenate([cosT, cosT], axis=0))
    sin128 = np.ascontiguousarray(np.concatenate([sinT, sinT], axis=0))

    perm = np.zeros((128, 128), np.float32)
    idx = np.arange(128)
    perm[idx ^ 1, idx] = 1.0
    eye64 = np.zeros((128, 64), np.float32)
    eye64[64:128, :] = np.eye(64, dtype=np.float32)
    u = np.arange(896)
    maskb = (u[None, :] >= (np.arange(128)[:, None] + 384)).astype(np.float32)

    in_maps = []
    for cid in range(8):
        b, h = cid // 4, cid % 4
        qcols = np.concatenate(
            [np.arange(g * 256 + h * 64, g * 256 + h * 64 + 64) for g in range(G)]
        )
        kcols = np.arange(1024 + h * 64, 1024 + h * 64 + 64)
        vcols = np.arange(1280 + h * 64, 1280 + h * 64 + 64)
        cols = np.concatenate([qcols, kcols, vcols])
        w_loc = W[:, cols].copy()
        b_loc = bb[cols].copy()
        w_loc[:, :256] *= SCALE
        b_loc[:256] *= SCALE
        b_loc_m = np.ascontiguousarray(b_loc.reshape(3, 128).T)  # (128, 3)

        rows = np.concatenate(
            [np.arange(g * 256 + h * 64, g * 256 + h * 64 + 64) for g in range(G)]
        )
        wf_loc = np.ascontiguousarray(Wf[rows, :])  # (256, 1024)
        bf_m = (
            np.ascontiguousarray(bf.reshape(8, 128).T)
            if h == 0
            else np.zeros((128, 8), np.float32)
        )

        in_maps.append(
            {
                "xT": np.ascontiguousarray(x[b].T),
                "w_qkv": w_loc,
                "b_loc": b_loc_m,
                "cosT": cos128,
                "sinT": sin128,
                "perm": perm,
                "eye64": eye64,
                "maskb": maskb,
                "wf": wf_loc,
                "bf": bf_m,
                "onesd": np.ones((128, 64), np.float32),
            }
        )
    return in_maps


def host_unshard(results):
    """Sum the 4 per-h partial yT per batch, transpose back to (B, T, C)."""
    out = np.empty((B, T, C), np.float32)
    for b in range(B):
        acc = results[b * 4]["yT"].astype(np.float32)
        for h in range(1, 4):
            acc = acc + results[b * 4 + h]["yT"]
        out[b] = acc.T
    return out


_NC_CACHE = None


def _get_nc():
    global _NC_CACHE
    if _NC_CACHE is None:
        _NC_CACHE = build_nc()
    return _NC_CACHE


def kernel(**inputs):
    nc = _get_nc()
    in_maps = host_shard(inputs)
    res = run_bass_kernel_spmd(nc, in_maps, core_ids=list(range(8)))
    return host_unshard(res.results)


# revision 14
# speedup vs baseline: 2.0187x; 1.0301x over previous
"""Trainium2 Bass kernel for grouped-query causal attention (B=2, T=2048, C=1024,
16 q heads / 4 kv heads, RoPE, fused qkv + output projection).

Sharding: 8 cores = (batch b, kv-head h). Each core:
  - projects x -> qT (4 heads), kT, vT with pre-sliced/pre-scaled weights
    (transposed layout: channels on partitions, T on free dim)
  - applies RoPE (pair-swap via permutation matmul on PE + DVE mul/add)
  - causal attention for its 4 query heads (S^T blocks, exp without
    max-subtraction [logits are O(8)], softmax denominators via a ones
    column appended to V, post-exp 0/1 causal mask)
  - partial output projection y^T = Wf_local^T @ oT  (transposed)
Host sums the 4 per-h partials per batch and transposes back.
"""

import sys

sys.path.insert(0, "/opt/trn_rl_repo")

import numpy as np

import concourse.bacc as bacc
import concourse.mybir as mybir
from concourse import tile
from concourse.bass_utils import run_bass_kernel_spmd

B, T, C = 2, 2048, 1024
G, HKV, HS = 4, 4, 64
OUT_DIM = C + 2 * (C // G)
SCALE = 1.0 / np.sqrt(HS)
MAX_PERIOD = 10000.0

F32 = mybir.dt.float32
F32R = mybir.dt.float32r
AF = mybir.ActivationFunctionType


TCH = T // 512  # 4 chunks of 512 along T
NT = T // 128  # 16 tiles of 128 along T


def build_nc():
    nc = bacc.Bacc(None, target_bir_lowering=False)

    xT_d = nc.dram_tensor("xT", [C, T], F32R, kind="ExternalInput")
    w_d = nc.dram_tensor("w_qkv", [C, 384], F32R, kind="ExternalInput")
    bl_d = nc.dram_tensor("b_loc", [128, 3], F32, kind="ExternalInput")
    cos_d = nc.dram_tensor("cosT", [128, T], F32R, kind="ExternalInput")
    sin_d = nc.dram_tensor("sinT", [128, T], F32R, kind="ExternalInput")
    perm_d = nc.dram_tensor("perm", [128, 128], F32R, kind="ExternalInput")
    eye_d = nc.dram_tensor("eye64", [128, 64], F32R, kind="ExternalInput")
    mask_d = nc.dram_tensor("maskb", [128, 896], F32R, kind="ExternalInput")
    wf_d = nc.dram_tensor("wf", [256, 1024], F32R, kind="ExternalInput")
    bf_d = nc.dram_tensor("bf", [128, 8], F32, kind="ExternalInput")
    ones_d = nc.dram_tensor("onesd", [128, 64], F32R, kind="ExternalInput")
    yT_d = nc.dram_tensor("yT", [C, T], F32, kind="ExternalOutput")

    with tile.TileContext(nc) as tc:
        with (
            tc.tile_pool(name="persist", bufs=1) as pp,
            tc.tile_pool(name="xstream", bufs=12) as spx,
            tc.tile_pool(name="pstream", bufs=6) as spp,
            tc.tile_pool(name="rstream", bufs=3) as spr,
            tc.tile_pool(name="ostream", bufs=3) as spo,
            tc.tile_pool(name="ps_acc", bufs=3, space="PSUM") as psacc,
            tc.tile_pool(name="ps_tmp", bufs=5, space="PSUM") as ps,
        ):
            # ---- persistent tiles ----
            w_sb = pp.tile([128, 8, 384], F32R, tag="w", name="w")
            bl_sb = pp.tile([128, 3], F32, tag="bl", name="bl")
            cos_sb = pp.tile([128, T], F32R, tag="cos", name="cos")
            sin_sb = pp.tile([128, T], F32R, tag="sin", name="sin")
            perm_sb = pp.tile([128, 128], F32R, tag="perm", name="perm")
            eye_sb = pp.tile([128, 64], F32R, tag="eye", name="eye")
            mask_sb = pp.tile([128, 896], F32R, tag="mask", name="mask")
            wf_sb = pp.tile([128, 2, 1024], F32R, tag="wf", name="wf")
            bf_sb = pp.tile([128, 8], F32, tag="bf", name="bf")
            ones_sb = pp.tile([128, 64], F32R, tag="ones", name="ones")
            qkvT = [pp.tile([128, T], F32R, tag=f"qkvT{m}", name=f"qkvT{m}") for m in range(3)]
            kdup = pp.tile([128, T], F32R, tag="kdup", name="kdup")
            v_sb = pp.tile([128, NT, 65], F32R, tag="vaug", name="vaug")
            oT_ab = [pp.tile([128, T], F32R, tag=f"oT{i}", name=f"oT{i}") for i in range(2)]

            nc.sync.dma_start(w_sb[:], w_d.rearrange("(k p) m -> p k m", p=128))
            nc.sync.dma_start(bl_sb[:], bl_d[:])
            nc.sync.dma_start(cos_sb[:], cos_d[:])
            nc.sync.dma_start(sin_sb[:], sin_d[:])
            nc.sync.dma_start(perm_sb[:], perm_d[:])
            nc.sync.dma_start(eye_sb[:], eye_d[:])
            nc.sync.dma_start(mask_sb[:], mask_d[:])
            nc.sync.dma_start(wf_sb[:], wf_d.rearrange("(c p) n -> p c n", p=128))
            nc.sync.dma_start(bf_sb[:], bf_d[:])
            nc.sync.dma_start(ones_sb[:], ones_d[:])
            nc.sync.dma_start(v_sb[:, :, 64:65], ones_d[:, 0:NT])

            # ---- phase 1+2: qkv^T projection, bias, RoPE, v transpose ----
            for tci in range(TCH):
                tsl = slice(tci * 512, (tci + 1) * 512)
                xts = []
                for k in range(8):
                    xt = spx.tile([128, 512], F32R, tag="xt", name="xt")
                    nc.sync.dma_start(xt[:], xT_d[k * 128 : (k + 1) * 128, tsl])
                    xts.append(xt)
                for mt in range(3):
                    pr = ps.tile([128, 512], F32, tag="tmp", name="tmp")
                    for k in range(8):
                        nc.tensor.matmul(
                            pr[:],
                            (w_sb[:, k, mt * 128 : (mt + 1) * 128]),
                            (xts[k][:]),
                            start=(k == 0),
                            stop=(k == 7),
                        )
                    nc.vector.tensor_scalar_add(
                        qkvT[mt][:, tsl], pr[:], bl_sb[:, mt : mt + 1]
                    )
                # RoPE on q tiles (all 128 partitions = 2 heads each)
                for mt in range(2):
                    tmp = ps.tile([128, 512], F32, tag="tmp", name="tmp")
                    nc.tensor.matmul(
                        tmp[:], (perm_sb[:]), (qkvT[mt][:, tsl]), start=True, stop=True
                    )
                    nc.vector.tensor_mul(
                        qkvT[mt][:, tsl], qkvT[mt][:, tsl], cos_sb[:, tsl]
                    )
                    nc.vector.tensor_mul(tmp[:], tmp[:], sin_sb[:, tsl])
                    nc.vector.tensor_add(qkvT[mt][:, tsl], qkvT[mt][:, tsl], tmp[:])
                # v transpose: (64, 128) blocks -> (128, 64) row-major v
                for i in range(4):
                    tt = tci * 4 + i
                    vt = ps.tile([128, 512], F32, tag="tmp", name="tmp")
                    nc.tensor.transpose(
                        vt[:, 0:64].bitcast(F32R),
                        (qkvT[2][64:128, tt * 128 : (tt + 1) * 128]),
                        (eye_sb[64:128, :]),
                    )
                    nc.vector.tensor_copy(v_sb[:, tt, 0:64], vt[:, 0:64])
                # RoPE on k rows (partitions 0..64 of tile 2)
                tmp = ps.tile([128, 512], F32, tag="tmp", name="tmp")
                nc.tensor.matmul(
                    tmp[0:64, :],
                    (perm_sb[:, 0:64]),
                    (qkvT[2][:, tsl]),
                    start=True,
                    stop=True,
                )
                nc.vector.tensor_mul(
                    qkvT[2][0:64, tsl], qkvT[2][0:64, tsl], cos_sb[0:64, tsl]
                )
                nc.vector.tensor_mul(tmp[0:64, :], tmp[0:64, :], sin_sb[0:64, tsl])
                nc.vector.tensor_add(
                    qkvT[2][0:64, tsl], qkvT[2][0:64, tsl], tmp[0:64, :]
                )
                # duplicate rotated k at partition base 64 for odd q-heads
                nc.sync.dma_start(kdup[64:128, tsl], qkvT[2][0:64, tsl])

            # ---- phase 3: attention per (head, tq-chunk) ----
            for g in range(G):
                qtile = qkvT[g // 2]
                qrow = (g % 2) * 64
                odd = g % 2 == 1
                for tci in range(TCH):
                    tsl = slice(tci * 512, (tci + 1) * 512)
                    nblk = 4 * tci + 4
                    o_ac = psacc.tile([128, 512], F32, tag="oacc", name="oacc")
                    DEPTH = 3
                    pq = []

                    def emit_pv(jj, p_tile):
                        nc.tensor.matmul(
                            o_ac[0:65, :],
                            (v_sb[:, jj, 0:65]),
                            (p_tile[:]),
                            start=(jj == 0),
                            stop=(jj == nblk - 1),
                        )

                    for j in range(nblk):
                        s_ps = ps.tile([128, 512], F32, tag="tmp", name="tmp")
                        ksrc = (
                            kdup[64:128, j * 128 : (j + 1) * 128]
                            if odd
                            else qkvT[2][0:64, j * 128 : (j + 1) * 128]
                        )
                        nc.tensor.matmul(
                            s_ps[:],
                            (ksrc),
                            (qtile[qrow : qrow + 64, tsl]),
                            start=True,
                            stop=True,
                        )
                        p_sb = spp.tile([128, 512], F32R, tag="p", name="p")
                        nc.scalar.activation(p_sb[:], s_ps[:], AF.Exp)
                        if j >= 4 * tci:
                            off = 384 + 512 * tci - 128 * j
                            nc.gpsimd.tensor_mul(
                                p_sb[:], p_sb[:], mask_sb[:, off : off + 512]
                            )
                        pq.append((j, p_sb))
                        if len(pq) > DEPTH:
                            emit_pv(*pq.pop(0))
                    for item in pq:
                        emit_pv(*item)
                    sums = spr.tile([128, 512], F32R, tag="rec", name="rec")
                    nc.vector.tensor_copy(sums[64:65, :], o_ac[64:65, :])
                    bc = ps.tile([128, 512], F32, tag="tmp", name="tmp")
                    nc.tensor.matmul(
                        bc[0:64, :],
                        (ones_sb[64:65, 0:64]),
                        (sums[64:65, :]),
                        start=True,
                        stop=True,
                    )
                    bc_sb = spr.tile([64, 512], F32, tag="bcs", name="bcs")
                    nc.vector.reciprocal_approx_fast(out=bc_sb[:], in_=bc[0:64, :])
                    if odd:
                        stg = spr.tile([64, 512], F32R, tag="stg", name="stg")
                        nc.vector.tensor_mul(stg[:], o_ac[0:64, :], bc_sb[:])
                        nc.sync.dma_start(oT_ab[g // 2][64:128, tsl], stg[:])
                    else:
                        nc.vector.tensor_mul(
                            oT_ab[g // 2][0:64, tsl], o_ac[0:64, :], bc_sb[:]
                        )

            # ---- phase 4: partial final projection y^T = wf^T @ oT + bias ----
            for tci in range(TCH):
                tsl = slice(tci * 512, (tci + 1) * 512)
                for nt in range(8):
                    y_ps = ps.tile([128, 512], F32, tag="tmp", name="tmp")
                    for cc in range(2):
                        nc.tensor.matmul(
                            y_ps[:],
                            (wf_sb[:, cc, nt * 128 : (nt + 1) * 128]),
                            (oT_ab[cc][:, tsl]),
                            start=(cc == 0),
                            stop=(cc == 1),
                        )
                    y_sb = spo.tile([128, 512], F32, tag="yout", name="yout")
                    nc.vector.tensor_scalar_add(
                        y_sb[:], y_ps[:], bf_sb[:, nt : nt + 1]
                    )
                    nc.sync.dma_start(yT_d[nt * 128 : (nt + 1) * 128, tsl], y_sb[:])

    nc.compile()
    return nc


def host_shard(inputs):
    """Build the 8 per-core input maps from full inputs."""
    x = np.ascontiguousarray(np.asarray(inputs["input"], dtype=np.float32))
    W = np.asarray(inputs["W_attn"], dtype=np.float32)
    bb = np.asarray(inputs["b_attn"], dtype=np.float32)
    Wf = np.asarray(inputs["W_final"], dtype=np.float32)
    bf = np.asarray(inputs["b_final"], dtype=np.float32)

    half = HS // 2
    inv_freq = MAX_PERIOD ** (-np.arange(half, dtype=np.float32) / half)
    ang = np.arange(T, dtype=np.float32)[:, None] * inv_freq  # (T, 32)
    sin_t = np.sin(ang).astype(np.float32)
    cos_t = np.cos(ang).astype(np.float32)
    cosT = np.repeat(cos_t.T, 2, axis=0)  # (64, T): row d -> cos(t*f[d//2])
    sgn = np.where(np.arange(HS) % 2 == 0, -1.0, 1.0).astype(np.float32)
    sinT = np.repeat(sin_t.T, 2, axis=0) * sgn[:, None]
    cos128 = np.ascontiguousarray(np.concatenate([cosT, cosT], axis=0))
    sin128 = np.ascontiguousarray(np.concatenate([sinT, sinT], axis=0))

    perm = np.zeros((128, 128), np.float32)
    idx = np.arange(128)
    perm[idx ^ 1, idx] = 1.0
    eye64 = np.zeros((128, 64), np.float32)
    eye64[64:128, :] = np.eye(64, dtype=np.float32)
    u = np.arange(896)
    maskb = (u[None, :] >= (np.arange(128)[:, None] + 384)).astype(np.float32)

    in_maps = []
    for cid in range(8):
        b, h = cid // 4, cid % 4
        qcols = np.concatenate(
            [np.arange(g * 256 + h * 64, g * 256 + h * 64 + 64) for g in range(G)]
        )
        kcols = np.arange(1024 + h * 64, 1024 + h * 64 + 64)
        vcols = np.arange(1280 + h * 64, 1280 + h * 64 + 64)
        cols = np.concatenate([qcols, kcols, vcols])
        w_loc = W[:, cols].copy()
        b_loc = bb[cols].copy()
        w_loc[:, :256] *= SCALE
        b_loc[:256] *= SCALE
        b_loc_m = np.ascontiguousarray(b_loc.reshape(3, 128).T)  # (128, 3)

        rows = np.concatenate(
            [np.arange(g * 256 + h * 64, g * 256 + h * 64 + 64) for g in range(G)]
        )
        wf_loc = np.ascontiguousarray(Wf[rows, :])  # (256, 1024)
        bf_m = (
            np.ascontiguousarray(bf.reshape(8, 128).T)
            if h == 0
            else np.zeros((128, 8), np.float32)
        )

        in_maps.append(
            {
                "xT": np.ascontiguousarray(x[b].T),
                "w_qkv": w_loc,
                "b_loc": b_loc_m,
                "cosT": cos128,
                "sinT": sin128,
                "perm": perm,
                "eye64": eye64,
                "maskb": maskb,
                "wf": wf_loc,
                "bf": bf_m,
                "onesd": np.ones((128, 64), np.float32),
            }
        )
    return in_maps


def host_unshard(results):
    """Sum the 4 per-h partial yT per batch, transpose back to (B, T, C)."""
    out = np.empty((B, T, C), np.float32)
    for b in range(B):
        acc = results[b * 4]["yT"].astype(np.float32)
        for h in range(1, 4):
            acc = acc + results[b * 4 + h]["yT"]
        out[b] = acc.T
    return out


_NC_CACHE = None


def _get_nc():
    global _NC_CACHE
    if _NC_CACHE is None:
        _NC_CACHE = build_nc()
    return _NC_CACHE


def kernel(**inputs):
    nc = _get_nc()
    in_maps = host_shard(inputs)
    res = run_bass_kernel_spmd(nc, in_maps, core_ids=list(range(8)))
    return host_unshard(res.results)


# revision 15
# speedup vs baseline: 2.2181x; 1.0988x over previous
"""Trainium2 Bass kernel for grouped-query causal attention (B=2, T=2048, C=1024,
16 q heads / 4 kv heads, RoPE, fused qkv + output projection).

Sharding: 8 cores = (batch b, kv-head h). Each core:
  - projects x -> qT (4 heads), kT, vT with pre-sliced/pre-scaled weights
    (transposed layout: channels on partitions, T on free dim)
  - applies RoPE (pair-swap via permutation matmul on PE + DVE mul/add)
  - causal attention for its 4 query heads (S^T blocks, exp without
    max-subtraction [logits are O(8)], softmax denominators via a ones
    column appended to V, post-exp 0/1 causal mask)
  - partial output projection y^T = Wf_local^T @ oT  (transposed)
Host sums the 4 per-h partials per batch and transposes back.
"""

import sys

sys.path.insert(0, "/opt/trn_rl_repo")

import numpy as np

import concourse.bacc as bacc
import concourse.mybir as mybir
from concourse import tile
from concourse.bass_utils import run_bass_kernel_spmd

B, T, C = 2, 2048, 1024
G, HKV, HS = 4, 4, 64
OUT_DIM = C + 2 * (C // G)
SCALE = 1.0 / np.sqrt(HS)
MAX_PERIOD = 10000.0

F32 = mybir.dt.float32
F32R = mybir.dt.float32r
AF = mybir.ActivationFunctionType


TCH = T // 512  # 4 chunks of 512 along T
NT = T // 128  # 16 tiles of 128 along T


def build_nc():
    nc = bacc.Bacc(None, target_bir_lowering=False)

    xT_d = nc.dram_tensor("xT", [C, T], F32R, kind="ExternalInput")
    w_d = nc.dram_tensor("w_qkv", [C, 384], F32R, kind="ExternalInput")
    bl_d = nc.dram_tensor("b_loc", [128, 3], F32, kind="ExternalInput")
    cos_d = nc.dram_tensor("cosT", [128, T], F32R, kind="ExternalInput")
    sin_d = nc.dram_tensor("sinT", [128, T], F32R, kind="ExternalInput")
    perm_d = nc.dram_tensor("perm", [128, 128], F32R, kind="ExternalInput")
    eye_d = nc.dram_tensor("eye64", [128, 64], F32R, kind="ExternalInput")
    mask_d = nc.dram_tensor("maskb", [128, 896], F32R, kind="ExternalInput")
    wf_d = nc.dram_tensor("wf", [256, 1024], F32R, kind="ExternalInput")
    bf_d = nc.dram_tensor("bf", [128, 8], F32, kind="ExternalInput")
    ones_d = nc.dram_tensor("onesd", [128, 64], F32R, kind="ExternalInput")
    yT_d = nc.dram_tensor("yT", [C, T], F32, kind="ExternalOutput")

    with tile.TileContext(nc) as tc:
        with (
            tc.tile_pool(name="persist", bufs=1) as pp,
            tc.tile_pool(name="xstream", bufs=10) as spx,
            tc.tile_pool(name="pstream", bufs=10) as spp,
            tc.tile_pool(name="rstream", bufs=3) as spr,
            tc.tile_pool(name="ostream", bufs=3) as spo,
            tc.tile_pool(name="ps_acc", bufs=4, space="PSUM") as psacc,
            tc.tile_pool(name="ps_tmp", bufs=4, space="PSUM") as ps,
        ):
            # ---- persistent tiles ----
            w_sb = pp.tile([128, 8, 384], F32R, tag="w", name="w")
            bl_sb = pp.tile([128, 3], F32, tag="bl", name="bl")
            cos_sb = pp.tile([128, T], F32R, tag="cos", name="cos")
            sin_sb = pp.tile([128, T], F32R, tag="sin", name="sin")
            perm_sb = pp.tile([128, 128], F32R, tag="perm", name="perm")
            eye_sb = pp.tile([128, 64], F32R, tag="eye", name="eye")
            mask_sb = pp.tile([128, 896], F32R, tag="mask", name="mask")
            wf_sb = pp.tile([128, 2, 1024], F32R, tag="wf", name="wf")
            bf_sb = pp.tile([128, 8], F32, tag="bf", name="bf")
            ones_sb = pp.tile([128, 64], F32R, tag="ones", name="ones")
            qkvT = [pp.tile([128, T], F32R, tag=f"qkvT{m}", name=f"qkvT{m}") for m in range(3)]
            kdup = pp.tile([128, T], F32R, tag="kdup", name="kdup")
            v_sb = pp.tile([128, NT, 65], F32R, tag="vaug", name="vaug")
            oT_ab = [pp.tile([128, T], F32R, tag=f"oT{i}", name=f"oT{i}") for i in range(2)]

            nc.sync.dma_start(w_sb[:], w_d.rearrange("(k p) m -> p k m", p=128))
            nc.sync.dma_start(bl_sb[:], bl_d[:])
            nc.sync.dma_start(cos_sb[:], cos_d[:])
            nc.sync.dma_start(sin_sb[:], sin_d[:])
            nc.sync.dma_start(perm_sb[:], perm_d[:])
            nc.sync.dma_start(eye_sb[:], eye_d[:])
            nc.sync.dma_start(mask_sb[:], mask_d[:])
            nc.sync.dma_start(wf_sb[:], wf_d.rearrange("(c p) n -> p c n", p=128))
            nc.sync.dma_start(bf_sb[:], bf_d[:])
            nc.sync.dma_start(ones_sb[:], ones_d[:])
            nc.sync.dma_start(v_sb[:, :, 64:65], ones_d[:, 0:NT])

            # -- emission helpers ------------------------------------------
            def load_x(tc_i):
                """Prefetch the 8 xT chunks for t-chunk tc_i."""
                tsl = slice(tc_i * 512, (tc_i + 1) * 512)
                xts = []
                for k in range(8):
                    xt = spx.tile([128, 512], F32R, tag="xt", name="xt")
                    nc.sync.dma_start(xt[:], xT_d[k * 128 : (k + 1) * 128, tsl])
                    xts.append(xt)
                return xts

            def proj_tasks(tc_i, xts):
                """QKV^T projection + RoPE + v-transpose for chunk tc_i,
                as a list of single-PE-instruction closures."""
                tsl = slice(tc_i * 512, (tc_i + 1) * 512)
                tasks = []
                state = {}

                def mk_mm(mt, k):
                    def run():
                        if k == 0:
                            state[mt] = ps.tile([128, 512], F32, tag="tmp", name="tmp")
                        pr = state[mt]
                        nc.tensor.matmul(
                            pr[:],
                            w_sb[:, k, mt * 128 : (mt + 1) * 128],
                            xts[k][:],
                            start=(k == 0),
                            stop=(k == 7),
                        )
                        if k == 7:
                            nc.vector.tensor_scalar_add(
                                qkvT[mt][:, tsl], pr[:], bl_sb[:, mt : mt + 1]
                            )
                    return run

                for mt in range(3):
                    for k in range(8):
                        tasks.append(mk_mm(mt, k))

                def mk_vt(i):
                    def run():
                        tt = tc_i * 4 + i
                        vt = ps.tile([128, 512], F32, tag="tmp", name="tmp")
                        nc.tensor.transpose(
                            vt[:, 0:64].bitcast(F32R),
                            qkvT[2][64:128, tt * 128 : (tt + 1) * 128],
                            eye_sb[64:128, :],
                        )
                        nc.vector.tensor_copy(v_sb[:, tt, 0:64], vt[:, 0:64])
                    return run

                for i in range(4):
                    tasks.append(mk_vt(i))

                def mk_rope_q(mt):
                    def run():
                        tmp = ps.tile([128, 512], F32, tag="tmp", name="tmp")
                        nc.tensor.matmul(
                            tmp[:], perm_sb[:], qkvT[mt][:, tsl], start=True, stop=True
                        )
                        nc.vector.tensor_mul(
                            qkvT[mt][:, tsl], qkvT[mt][:, tsl], cos_sb[:, tsl]
                        )
                        nc.vector.tensor_mul(tmp[:], tmp[:], sin_sb[:, tsl])
                        nc.vector.tensor_add(qkvT[mt][:, tsl], qkvT[mt][:, tsl], tmp[:])
                    return run

                def rope_k():
                    tmp = ps.tile([128, 512], F32, tag="tmp", name="tmp")
                    nc.tensor.matmul(
                        tmp[0:64, :], perm_sb[:, 0:64], qkvT[2][:, tsl],
                        start=True, stop=True,
                    )
                    nc.vector.tensor_mul(
                        qkvT[2][0:64, tsl], qkvT[2][0:64, tsl], cos_sb[0:64, tsl]
                    )
                    nc.vector.tensor_mul(tmp[0:64, :], tmp[0:64, :], sin_sb[0:64, tsl])
                    nc.vector.tensor_add(
                        qkvT[2][0:64, tsl], qkvT[2][0:64, tsl], tmp[0:64, :]
                    )
                    nc.sync.dma_start(kdup[64:128, tsl], qkvT[2][0:64, tsl])

                tasks.append(mk_rope_q(0))
                tasks.append(mk_rope_q(1))
                tasks.append(rope_k)
                return tasks

            def final_tasks(tc_i):
                """Partial output projection for chunk tc_i (16 PE closures)."""
                tsl = slice(tc_i * 512, (tc_i + 1) * 512)
                tasks = []
                state = {}

                def mk(nt, cc):
                    def run():
                        if cc == 0:
                            state[nt] = ps.tile([128, 512], F32, tag="tmp", name="tmp")
                        y_ps = state[nt]
                        nc.tensor.matmul(
                            y_ps[:],
                            wf_sb[:, cc, nt * 128 : (nt + 1) * 128],
                            oT_ab[cc][:, tsl],
                            start=(cc == 0),
                            stop=(cc == 1),
                        )
                        if cc == 1:
                            y_sb = spo.tile([128, 512], F32, tag="yout", name="yout")
                            nc.vector.tensor_scalar_add(
                                y_sb[:], y_ps[:], bf_sb[:, nt : nt + 1]
                            )
                            nc.sync.dma_start(
                                yT_d[nt * 128 : (nt + 1) * 128, tsl], y_sb[:]
                            )
                    return run

                for nt in range(8):
                    for cc in range(2):
                        tasks.append(mk(nt, cc))
                return tasks

            # -- prologue: load + project chunk 0 --------------------------
            xts0 = load_x(0)
            for t in proj_tasks(0, xts0):
                t()

            # -- main loop: attention(tci) with background work stuffed in -
            for tci in range(TCH):
                tsl = slice(tci * 512, (tci + 1) * 512)
                nblk = 4 * tci + 4

                bg = []
                if tci + 1 < TCH:
                    xts = load_x(tci + 1)
                    bg += proj_tasks(tci + 1, xts)
                if tci >= 1:
                    bg += final_tasks(tci - 1)
                bg_done = 0
                bg_total = len(bg)

                o_acs = [
                    psacc.tile([128, 512], F32, tag="oacc", name="oacc")
                    for _ in range(G)
                ]
                pend = [None] * G  # p tile awaiting PV, per head

                def emit_pv(g, jj, p_tile):
                    nc.tensor.matmul(
                        o_acs[g][0:65, :],
                        v_sb[:, jj, 0:65],
                        p_tile[:],
                        start=(jj == 0),
                        stop=(jj == nblk - 1),
                    )

                slots = nblk * G
                slot = 0
                for j in range(nblk):
                    for g in range(G):
                        odd = g % 2 == 1
                        qtile = qkvT[g // 2]
                        qrow = (g % 2) * 64
                        s_ps = ps.tile([128, 512], F32, tag="tmp", name="tmp")
                        ksrc = (
                            kdup[64:128, j * 128 : (j + 1) * 128]
                            if odd
                            else qkvT[2][0:64, j * 128 : (j + 1) * 128]
                        )
                        nc.tensor.matmul(
                            s_ps[:], ksrc, qtile[qrow : qrow + 64, tsl],
                            start=True, stop=True,
                        )
                        p_sb = spp.tile([128, 512], F32R, tag="p", name="p")
                        nc.scalar.activation(p_sb[:], s_ps[:], AF.Exp)
                        if j >= 4 * tci:
                            off = 384 + 512 * tci - 128 * j
                            nc.gpsimd.tensor_mul(
                                p_sb[:], p_sb[:], mask_sb[:, off : off + 512]
                            )
                        if pend[g] is not None:
                            emit_pv(g, j - 1, pend[g])
                        pend[g] = p_sb
                        slot += 1
                        # stuff background PE work evenly across the window
                        due = bg_total * slot // slots
                        while bg_done < due:
                            bg[bg_done]()
                            bg_done += 1

                while bg_done < bg_total:
                    bg[bg_done]()
                    bg_done += 1

                # tail: last PVs + softmax normalization per head
                for g in range(G):
                    odd = g % 2 == 1
                    emit_pv(g, nblk - 1, pend[g])
                    o_ac = o_acs[g]
                    sums = spr.tile([128, 512], F32R, tag="rec", name="rec")
                    nc.vector.tensor_copy(sums[64:65, :], o_ac[64:65, :])
                    bc = ps.tile([128, 512], F32, tag="tmp", name="tmp")
                    nc.tensor.matmul(
                        bc[0:64, :], ones_sb[64:65, 0:64], sums[64:65, :],
                        start=True, stop=True,
                    )
                    bc_sb = spr.tile([64, 512], F32, tag="bcs", name="bcs")
                    nc.vector.reciprocal_approx_fast(out=bc_sb[:], in_=bc[0:64, :])
                    if odd:
                        stg = spr.tile([64, 512], F32R, tag="stg", name="stg")
                        nc.vector.tensor_mul(stg[:], o_ac[0:64, :], bc_sb[:])
                        nc.sync.dma_start(oT_ab[g // 2][64:128, tsl], stg[:])
                    else:
                        nc.vector.tensor_mul(
                            oT_ab[g // 2][0:64, tsl], o_ac[0:64, :], bc_sb[:]
                        )

            # -- epilogue: final projection for the last chunk -------------
            for t in final_tasks(TCH - 1):
                t()

    nc.compile()
    return nc


def host_shard(inputs):
    """Build the 8 per-core input maps from full inputs."""
    x = np.ascontiguousarray(np.asarray(inputs["input"], dtype=np.float32))
    W = np.asarray(inputs["W_attn"], dtype=np.float32)
    bb = np.asarray(inputs["b_attn"], dtype=np.float32)
    Wf = np.asarray(inputs["W_final"], dtype=np.float32)
    bf = np.asarray(inputs["b_final"], dtype=np.float32)

    half = HS // 2
    inv_freq = MAX_PERIOD ** (-np.arange(half, dtype=np.float32) / half)
    ang = np.arange(T, dtype=np.float32)[:, None] * inv_freq  # (T, 32)
    sin_t = np.sin(ang).astype(np.float32)
    cos_t = np.cos(ang).astype(np.float32)
    cosT = np.repeat(cos_t.T, 2, axis=0)  # (64, T): row d -> cos(t*f[d//2])
    sgn = np.where(np.arange(HS) % 2 == 0, -1.0, 1.0).astype(np.float32)
    sinT = np.repeat(sin_t.T, 2, axis=0) * sgn[:, None]
    cos128 = np.ascontiguousarray(np.concatenate([cosT, cosT], axis=0))
    sin128 = np.ascontiguousarray(np.concatenate([sinT, sinT], axis=0))

    perm = np.zeros((128, 128), np.float32)
    idx = np.arange(128)
    perm[idx ^ 1, idx] = 1.0
    eye64 = np.zeros((128, 64), np.float32)
    eye64[64:128, :] = np.eye(64, dtype=np.float32)
    u = np.arange(896)
    maskb = (u[None, :] >= (np.arange(128)[:, None] + 384)).astype(np.float32)

    in_maps = []
    for cid in range(8):
        b, h = cid // 4, cid % 4
        qcols = np.concatenate(
            [np.arange(g * 256 + h * 64, g * 256 + h * 64 + 64) for g in range(G)]
        )
        kcols = np.arange(1024 + h * 64, 1024 + h * 64 + 64)
        vcols = np.arange(1280 + h * 64, 1280 + h * 64 + 64)
        cols = np.concatenate([qcols, kcols, vcols])
        w_loc = W[:, cols].copy()
        b_loc = bb[cols].copy()
        w_loc[:, :256] *= SCALE
        b_loc[:256] *= SCALE
        b_loc_m = np.ascontiguousarray(b_loc.reshape(3, 128).T)  # (128, 3)

        rows = np.concatenate(
            [np.arange(g * 256 + h * 64, g * 256 + h * 64 + 64) for g in range(G)]
        )
        wf_loc = np.ascontiguousarray(Wf[rows, :])  # (256, 1024)
        bf_m = (
            np.ascontiguousarray(bf.reshape(8, 128).T)
            if h == 0
            else np.zeros((128, 8), np.float32)
        )

        in_maps.append(
            {
                "xT": np.ascontiguousarray(x[b].T),
                "w_qkv": w_loc,
                "b_loc": b_loc_m,
                "cosT": cos128,
                "sinT": sin128,
                "perm": perm,
                "eye64": eye64,
                "maskb": maskb,
                "wf": wf_loc,
                "bf": bf_m,
                "onesd": np.ones((128, 64), np.float32),
            }
        )
    return in_maps


def host_unshard(results):
    """Sum the 4 per-h partial yT per batch, transpose back to (B, T, C)."""
    out = np.empty((B, T, C), np.float32)
    for b in range(B):
        acc = results[b * 4]["yT"].astype(np.float32)
        for h in range(1, 4):
            acc = acc + results[b * 4 + h]["yT"]
        out[b] = acc.T
    return out


_NC_CACHE = None


def _get_nc():
    global _NC_CACHE
    if _NC_CACHE is None:
        _NC_CACHE = build_nc()
    return _NC_CACHE


def kernel(**inputs):
    nc = _get_nc()
    in_maps = host_shard(inputs)
    res = run_bass_kernel_spmd(nc, in_maps, core_ids=list(range(8)))
    return host_unshard(res.results)


# revision 16
# speedup vs baseline: 2.3067x; 1.0399x over previous
"""Trainium2 Bass kernel for grouped-query causal attention (B=2, T=2048, C=1024,
16 q heads / 4 kv heads, RoPE, fused qkv + output projection).

Sharding: 8 cores = (batch b, kv-head h). Each core:
  - projects x -> qT (4 heads), kT, vT with pre-sliced/pre-scaled weights
    (transposed layout: channels on partitions, T on free dim)
  - applies RoPE (pair-swap via permutation matmul on PE + DVE mul/add)
  - causal attention for its 4 query heads (S^T blocks, exp without
    max-subtraction [logits are O(8)], softmax denominators via a ones
    column appended to V, post-exp 0/1 causal mask)
  - partial output projection y^T = Wf_local^T @ oT  (transposed)
Host sums the 4 per-h partials per batch and transposes back.
"""

import sys

sys.path.insert(0, "/opt/trn_rl_repo")

import numpy as np

import concourse.bacc as bacc
import concourse.mybir as mybir
from concourse import tile
from concourse.bass_utils import run_bass_kernel_spmd

B, T, C = 2, 2048, 1024
G, HKV, HS = 4, 4, 64
OUT_DIM = C + 2 * (C // G)
SCALE = 1.0 / np.sqrt(HS)
MAX_PERIOD = 10000.0

F32 = mybir.dt.float32
F32R = mybir.dt.float32r
AF = mybir.ActivationFunctionType


TCH = T // 512  # 4 chunks of 512 along T
NT = T // 128  # 16 tiles of 128 along T


def build_nc():
    nc = bacc.Bacc(None, target_bir_lowering=False)

    xT_d = nc.dram_tensor("xT", [C, T], F32R, kind="ExternalInput")
    w_d = nc.dram_tensor("w_qkv", [C, 384], F32R, kind="ExternalInput")
    bl_d = nc.dram_tensor("b_loc", [128, 3], F32, kind="ExternalInput")
    cos_d = nc.dram_tensor("cosT", [128, T], F32R, kind="ExternalInput")
    sin_d = nc.dram_tensor("sinT", [128, T], F32R, kind="ExternalInput")
    perm_d = nc.dram_tensor("perm", [128, 128], F32R, kind="ExternalInput")
    eye_d = nc.dram_tensor("eye64", [128, 64], F32R, kind="ExternalInput")
    mask_d = nc.dram_tensor("maskb", [128, 896], F32R, kind="ExternalInput")
    wf_d = nc.dram_tensor("wf", [256, 1024], F32R, kind="ExternalInput")
    bf_d = nc.dram_tensor("bf", [128, 8], F32, kind="ExternalInput")
    ones_d = nc.dram_tensor("onesd", [128, 64], F32R, kind="ExternalInput")
    yT_d = nc.dram_tensor("yT", [C, T], F32, kind="ExternalOutput")

    with tile.TileContext(nc) as tc:
        with (
            tc.tile_pool(name="persist", bufs=1) as pp,
            tc.tile_pool(name="xstream", bufs=10) as spx,
            tc.tile_pool(name="pstream", bufs=8) as spp,
            tc.tile_pool(name="rstream", bufs=3) as spr,
            tc.tile_pool(name="ostream", bufs=3) as spo,
            tc.tile_pool(name="ps_acc", bufs=2, space="PSUM") as psacc,
            tc.tile_pool(name="ps_s", bufs=4, space="PSUM") as pss,
            tc.tile_pool(name="ps_tmp", bufs=2, space="PSUM") as ps,
        ):
            # ---- persistent tiles ----
            w_sb = pp.tile([128, 8, 384], F32R, tag="w", name="w")
            bl_sb = pp.tile([128, 3], F32, tag="bl", name="bl")
            cos_sb = pp.tile([128, T], F32R, tag="cos", name="cos")
            sin_sb = pp.tile([128, T], F32R, tag="sin", name="sin")
            perm_sb = pp.tile([128, 128], F32R, tag="perm", name="perm")
            eye_sb = pp.tile([128, 64], F32R, tag="eye", name="eye")
            mask_sb = pp.tile([128, 896], F32R, tag="mask", name="mask")
            wf_sb = pp.tile([128, 2, 1024], F32R, tag="wf", name="wf")
            bf_sb = pp.tile([128, 8], F32, tag="bf", name="bf")
            ones_sb = pp.tile([128, 64], F32R, tag="ones", name="ones")
            qkvT = [pp.tile([128, T], F32R, tag=f"qkvT{m}", name=f"qkvT{m}") for m in range(3)]
            kdup = pp.tile([128, T], F32R, tag="kdup", name="kdup")
            v_sb = pp.tile([128, NT, 65], F32R, tag="vaug", name="vaug")
            oT_ab = [pp.tile([128, T], F32R, tag=f"oT{i}", name=f"oT{i}") for i in range(2)]

            nc.sync.dma_start(w_sb[:], w_d.rearrange("(k p) m -> p k m", p=128))
            nc.sync.dma_start(bl_sb[:], bl_d[:])
            nc.sync.dma_start(cos_sb[:], cos_d[:])
            nc.sync.dma_start(sin_sb[:], sin_d[:])
            nc.sync.dma_start(perm_sb[:], perm_d[:])
            nc.sync.dma_start(eye_sb[:], eye_d[:])
            nc.sync.dma_start(mask_sb[:], mask_d[:])
            nc.sync.dma_start(wf_sb[:], wf_d.rearrange("(c p) n -> p c n", p=128))
            nc.sync.dma_start(bf_sb[:], bf_d[:])
            nc.sync.dma_start(ones_sb[:], ones_d[:])
            nc.sync.dma_start(v_sb[:, :, 64:65], ones_d[:, 0:NT])

            # -- emission helpers ------------------------------------------
            def load_x(tc_i):
                tsl = slice(tc_i * 512, (tc_i + 1) * 512)
                xts = []
                for k in range(8):
                    xt = spx.tile([128, 512], F32R, tag="xt", name="xt")
                    nc.sync.dma_start(xt[:], xT_d[k * 128 : (k + 1) * 128, tsl])
                    xts.append(xt)
                return xts

            def proj_tasks(tc_i, xts):
                """Dense background tasks for chunk tc_i's projection+RoPE+vT."""
                tsl = slice(tc_i * 512, (tc_i + 1) * 512)

                def mk_group(mt):
                    def run():
                        pr = ps.tile([128, 512], F32, tag="tmp", name="tmp")
                        for k in range(8):
                            nc.tensor.matmul(
                                pr[:],
                                w_sb[:, k, mt * 128 : (mt + 1) * 128],
                                xts[k][:],
                                start=(k == 0),
                                stop=(k == 7),
                            )
                        nc.vector.tensor_scalar_add(
                            qkvT[mt][:, tsl], pr[:], bl_sb[:, mt : mt + 1]
                        )
                    return run

                def mk_rope_q(mt):
                    def run():
                        tmp = ps.tile([128, 512], F32, tag="tmp", name="tmp")
                        nc.tensor.matmul(
                            tmp[:], perm_sb[:], qkvT[mt][:, tsl], start=True, stop=True
                        )
                        nc.vector.tensor_mul(
                            qkvT[mt][:, tsl], qkvT[mt][:, tsl], cos_sb[:, tsl]
                        )
                        nc.vector.tensor_mul(tmp[:], tmp[:], sin_sb[:, tsl])
                        nc.vector.tensor_add(qkvT[mt][:, tsl], qkvT[mt][:, tsl], tmp[:])
                    return run

                def mk_vt(i):
                    def run():
                        tt = tc_i * 4 + i
                        vt = ps.tile([128, 512], F32, tag="tmp", name="tmp")
                        nc.tensor.transpose(
                            vt[:, 0:64].bitcast(F32R),
                            qkvT[2][64:128, tt * 128 : (tt + 1) * 128],
                            eye_sb[64:128, :],
                        )
                        nc.vector.tensor_copy(v_sb[:, tt, 0:64], vt[:, 0:64])
                    return run

                def rope_k():
                    tmp = ps.tile([128, 512], F32, tag="tmp", name="tmp")
                    nc.tensor.matmul(
                        tmp[0:64, :], perm_sb[:, 0:64], qkvT[2][:, tsl],
                        start=True, stop=True,
                    )
                    nc.vector.tensor_mul(
                        qkvT[2][0:64, tsl], qkvT[2][0:64, tsl], cos_sb[0:64, tsl]
                    )
                    nc.vector.tensor_mul(tmp[0:64, :], tmp[0:64, :], sin_sb[0:64, tsl])
                    nc.vector.tensor_add(
                        qkvT[2][0:64, tsl], qkvT[2][0:64, tsl], tmp[0:64, :]
                    )
                    nc.sync.dma_start(kdup[64:128, tsl], qkvT[2][0:64, tsl])

                return [
                    mk_group(0), mk_group(1), mk_group(2),
                    mk_rope_q(0), mk_rope_q(1),
                    mk_vt(0), mk_vt(1), mk_vt(2), mk_vt(3),
                    rope_k,
                ]

            def final_tasks(tc_i):
                tsl = slice(tc_i * 512, (tc_i + 1) * 512)

                def mk(nt):
                    def run():
                        y_ps = ps.tile([128, 512], F32, tag="tmp", name="tmp")
                        for cc in range(2):
                            nc.tensor.matmul(
                                y_ps[:],
                                wf_sb[:, cc, nt * 128 : (nt + 1) * 128],
                                oT_ab[cc][:, tsl],
                                start=(cc == 0),
                                stop=(cc == 1),
                            )
                        y_sb = spo.tile([128, 512], F32, tag="yout", name="yout")
                        nc.vector.tensor_scalar_add(
                            y_sb[:], y_ps[:], bf_sb[:, nt : nt + 1]
                        )
                        nc.sync.dma_start(yT_d[nt * 128 : (nt + 1) * 128, tsl], y_sb[:])
                    return run

                return [mk(nt) for nt in range(8)]

            # -- prologue --------------------------------------------------
            xts0 = load_x(0)
            for t in proj_tasks(0, xts0):
                t()

            # -- main loop -------------------------------------------------
            pending_norm = []  # deferred norm-finish closures

            def mk_norm(g, tci, o_ac, sums):
                tsl = slice(tci * 512, (tci + 1) * 512)
                odd = g % 2 == 1

                def run():
                    bc = ps.tile([128, 512], F32, tag="tmp", name="tmp")
                    nc.tensor.matmul(
                        bc[0:64, :], ones_sb[64:65, 0:64], sums[64:65, :],
                        start=True, stop=True,
                    )
                    bc_sb = spr.tile([64, 512], F32, tag="bcs", name="bcs")
                    nc.vector.reciprocal_approx_fast(out=bc_sb[:], in_=bc[0:64, :])
                    if odd:
                        stg = spr.tile([64, 512], F32R, tag="stg", name="stg")
                        nc.vector.tensor_mul(stg[:], o_ac[0:64, :], bc_sb[:])
                        nc.sync.dma_start(oT_ab[g // 2][64:128, tsl], stg[:])
                    else:
                        nc.vector.tensor_mul(
                            oT_ab[g // 2][0:64, tsl], o_ac[0:64, :], bc_sb[:]
                        )
                return run

            for tci in range(TCH):
                tsl = slice(tci * 512, (tci + 1) * 512)
                nblk = 4 * tci + 4

                bg = []
                if tci + 1 < TCH:
                    xts = load_x(tci + 1)
                    bg += proj_tasks(tci + 1, xts)
                if tci == 2:
                    bg += final_tasks(0)
                elif tci == 3:
                    bg += final_tasks(1) + final_tasks(2)
                bg_done = 0
                bg_total = len(bg)
                slots = G * nblk
                slot = 0

                for g in range(G):
                    odd = g % 2 == 1
                    qtile = qkvT[g // 2]
                    qrow = (g % 2) * 64
                    o_ac = psacc.tile([128, 512], F32, tag="oacc", name="oacc")
                    DEPTH = 3
                    pq = []

                    def emit_pv(jj, p_tile, o_ac=o_ac, nblk=nblk):
                        nc.tensor.matmul(
                            o_ac[0:65, :],
                            v_sb[:, jj, 0:65],
                            p_tile[:],
                            start=(jj == 0),
                            stop=(jj == nblk - 1),
                        )

                    for j in range(nblk):
                        s_ps = pss.tile([128, 512], F32, tag="s", name="s")
                        ksrc = (
                            kdup[64:128, j * 128 : (j + 1) * 128]
                            if odd
                            else qkvT[2][0:64, j * 128 : (j + 1) * 128]
                        )
                        nc.tensor.matmul(
                            s_ps[:], ksrc, qtile[qrow : qrow + 64, tsl],
                            start=True, stop=True,
                        )
                        p_sb = spp.tile([128, 512], F32R, tag="p", name="p")
                        nc.scalar.activation(p_sb[:], s_ps[:], AF.Exp)
                        if j >= 4 * tci:
                            off = 384 + 512 * tci - 128 * j
                            nc.gpsimd.tensor_mul(
                                p_sb[:], p_sb[:], mask_sb[:, off : off + 512]
                            )
                        pq.append((j, p_sb))
                        if len(pq) > DEPTH:
                            emit_pv(*pq.pop(0))
                        # drain deferred norms and background work
                        if pending_norm and slot % 2 == 1:
                            pending_norm.pop(0)()
                        slot += 1
                        due = bg_total * slot // slots
                        while bg_done < due:
                            bg[bg_done]()
                            bg_done += 1

                    for item in pq:
                        emit_pv(*item)
                    sums = spr.tile([128, 512], F32R, tag="rec", name="rec")
                    nc.vector.tensor_copy(sums[64:65, :], o_ac[64:65, :])
                    pending_norm.append(mk_norm(g, tci, o_ac, sums))

                while bg_done < bg_total:
                    bg[bg_done]()
                    bg_done += 1

            for fn in pending_norm:
                fn()
            for t in final_tasks(TCH - 1):
                t()

    nc.compile()
    return nc


def host_shard(inputs):
    """Build the 8 per-core input maps from full inputs."""
    x = np.ascontiguousarray(np.asarray(inputs["input"], dtype=np.float32))
    W = np.asarray(inputs["W_attn"], dtype=np.float32)
    bb = np.asarray(inputs["b_attn"], dtype=np.float32)
    Wf = np.asarray(inputs["W_final"], dtype=np.float32)
    bf = np.asarray(inputs["b_final"], dtype=np.float32)

    half = HS // 2
    inv_freq = MAX_PERIOD ** (-np.arange(half, dtype=np.float32) / half)
    ang = np.arange(T, dtype=np.float32)[:, None] * inv_freq  # (T, 32)
    sin_t = np.sin(ang).astype(np.float32)
    cos_t = np.cos(ang).astype(np.float32)
    cosT = np.repeat(cos_t.T, 2, axis=0)  # (64, T): row d -> cos(t*f[d//2])
    sgn = np.where(np.arange(HS) % 2 == 0, -1.0, 1.0).astype(np.float32)
    sinT = np.repeat(sin_t.T, 2, axis=0) * sgn[:, None]
    cos128 = np.ascontiguousarray(np.concatenate([cosT, cosT], axis=0))
    sin128 = np.ascontiguousarray(np.concatenate([sinT, sinT], axis=0))

    perm = np.zeros((128, 128), np.float32)
    idx = np.arange(128)
    perm[idx ^ 1, idx] = 1.0
    eye64 = np.zeros((128, 64), np.float32)
    eye64[64:128, :] = np.eye(64, dtype=np.float32)
    u = np.arange(896)
    maskb = (u[None, :] >= (np.arange(128)[:, None] + 384)).astype(np.float32)

    in_maps = []
    for cid in range(8):
        b, h = cid // 4, cid % 4
        qcols = np.concatenate(
            [np.arange(g * 256 + h * 64, g * 256 + h * 64 + 64) for g in range(G)]
        )
        kcols = np.arange(1024 + h * 64, 1024 + h * 64 + 64)
        vcols = np.arange(1280 + h * 64, 1280 + h * 64 + 64)
        cols = np.concatenate([qcols, kcols, vcols])
        w_loc = W[:, cols].copy()
        b_loc = bb[cols].copy()
        w_loc[:, :256] *= SCALE
        b_loc[:256] *= SCALE
        b_loc_m = np.ascontiguousarray(b_loc.reshape(3, 128).T)  # (128, 3)

        rows = np.concatenate(
            [np.arange(g * 256 + h * 64, g * 256 + h * 64 + 64) for g in range(G)]
        )
        wf_loc = np.ascontiguousarray(Wf[rows, :])  # (256, 1024)
        bf_m = (
            np.ascontiguousarray(bf.reshape(8, 128).T)
            if h == 0
            else np.zeros((128, 8), np.float32)
        )

        in_maps.append(
            {
                "xT": np.ascontiguousarray(x[b].T),
                "w_qkv": w_loc,
                "b_loc": b_loc_m,
                "cosT": cos128,
                "sinT": sin128,
                "perm": perm,
                "eye64": eye64,
                "maskb": maskb,
                "wf": wf_loc,
                "bf": bf_m,
                "onesd": np.ones((128, 64), np.float32),
            }
        )
    return in_maps


def host_unshard(results):
    """Sum the 4 per-h partial yT per batch, transpose back to (B, T, C)."""
    out = np.empty((B, T, C), np.float32)
    for b in range(B):
        acc = results[b * 4]["yT"].astype(np.float32)
        for h in range(1, 4):
            acc = acc + results[b * 4 + h]["yT"]
        out[b] = acc.T
    return out


_NC_CACHE = None


def _get_nc():
    global _NC_CACHE
    if _NC_CACHE is None:
        _NC_CACHE = build_nc()
    return _NC_CACHE


def kernel(**inputs):
    nc = _get_nc()
    in_maps = host_shard(inputs)
    res = run_bass_kernel_spmd(nc, in_maps, core_ids=list(range(8)))
    return host_unshard(res.results)


# revision 19
# speedup vs baseline: 2.5434x; 1.1026x over previous
"""Trainium2 Bass kernel for grouped-query causal attention (B=2, T=2048, C=1024,
16 q heads / 4 kv heads, RoPE, fused qkv + output projection).

Sharding: 8 cores = (batch b, kv-head h). Each core:
  - projects x -> qT (4 heads), kT, vT with pre-sliced/pre-scaled weights
    (transposed layout: channels on partitions, T on free dim)
  - applies RoPE (pair-swap via permutation matmul on PE + DVE mul/add)
  - causal attention for its 4 query heads (S^T blocks, exp without
    max-subtraction [logits are O(8)], softmax denominators via a ones
    column appended to V, post-exp 0/1 causal mask)
  - partial output projection y^T = Wf_local^T @ oT  (transposed)
Host sums the 4 per-h partials per batch and transposes back.
"""

import sys

sys.path.insert(0, "/opt/trn_rl_repo")

import ml_dtypes
import numpy as np

import concourse.bacc as bacc
import concourse.mybir as mybir
from concourse import tile
from concourse.bass_utils import run_bass_kernel_spmd

B, T, C = 2, 2048, 1024
G, HKV, HS = 4, 4, 64
OUT_DIM = C + 2 * (C // G)
SCALE = 1.0 / np.sqrt(HS)
MAX_PERIOD = 10000.0

F32 = mybir.dt.float32
F32R = mybir.dt.float32r
BF16 = mybir.dt.bfloat16
AF = mybir.ActivationFunctionType


TCH = T // 512  # 4 chunks of 512 along T
NT = T // 128  # 16 tiles of 128 along T


def build_nc():
    nc = bacc.Bacc(None, target_bir_lowering=False)

    xT_d = nc.dram_tensor("xT", [C, T], F32R, kind="ExternalInput")
    w_d = nc.dram_tensor("w_qkv", [C, 384], F32R, kind="ExternalInput")
    bl_d = nc.dram_tensor("b_loc", [128, 3], F32, kind="ExternalInput")
    cos_d = nc.dram_tensor("cosT", [128, T], BF16, kind="ExternalInput")
    sin_d = nc.dram_tensor("sinT", [128, T], BF16, kind="ExternalInput")
    perm_d = nc.dram_tensor("perm", [128, 128], BF16, kind="ExternalInput")
    eye_d = nc.dram_tensor("eye64", [128, 64], BF16, kind="ExternalInput")
    mask_d = nc.dram_tensor("maskb", [128, 896], BF16, kind="ExternalInput")
    wf_d = nc.dram_tensor("wf", [256, 1024], F32R, kind="ExternalInput")
    bf_d = nc.dram_tensor("bf", [128, 8], F32, kind="ExternalInput")
    ones_d = nc.dram_tensor("onesd", [128, 64], F32R, kind="ExternalInput")
    yT_d = nc.dram_tensor("yT", [C, T], F32, kind="ExternalOutput")

    with tile.TileContext(nc) as tc:
        with (
            tc.tile_pool(name="persist", bufs=1) as pp,
            tc.tile_pool(name="xstream", bufs=10) as spx,
            tc.tile_pool(name="pstream", bufs=8) as spp,
            tc.tile_pool(name="rstream", bufs=3) as spr,
            tc.tile_pool(name="ostream", bufs=3) as spo,
            tc.tile_pool(name="ps_acc", bufs=2, space="PSUM") as psacc,
            tc.tile_pool(name="ps_s", bufs=3, space="PSUM") as pss,
            tc.tile_pool(name="ps_tmp", bufs=2, space="PSUM") as ps,
        ):
            # ---- persistent tiles ----
            w_sb = pp.tile([128, 8, 384], F32R, tag="w", name="w")
            bl_sb = pp.tile([128, 3], F32, tag="bl", name="bl")
            cos_sb = pp.tile([128, T], BF16, tag="cos", name="cos")
            sin_sb = pp.tile([128, T], BF16, tag="sin", name="sin")
            perm_sb = pp.tile([128, 128], BF16, tag="perm", name="perm")
            eye_sb = pp.tile([128, 64], BF16, tag="eye", name="eye")
            mask_sb = pp.tile([128, 896], BF16, tag="mask", name="mask")
            wf_sb = pp.tile([128, 2, 1024], F32R, tag="wf", name="wf")
            bf_sb = pp.tile([128, 8], F32, tag="bf", name="bf")
            ones_sb = pp.tile([128, 64], F32R, tag="ones", name="ones")
            qkvT = [pp.tile([128, T], BF16, tag=f"qkvT{m}", name=f"qkvT{m}") for m in range(3)]
            kdup = pp.tile([128, T], BF16, tag="kdup", name="kdup")
            v_sb = pp.tile([128, NT, 65], BF16, tag="vaug", name="vaug")
            oT_ab = [pp.tile([128, T], F32R, tag=f"oT{i}", name=f"oT{i}") for i in range(2)]

            nc.sync.dma_start(w_sb[:], w_d.rearrange("(k p) m -> p k m", p=128))
            nc.sync.dma_start(bl_sb[:], bl_d[:])
            nc.sync.dma_start(cos_sb[:], cos_d[:])
            nc.sync.dma_start(sin_sb[:], sin_d[:])
            nc.sync.dma_start(perm_sb[:], perm_d[:])
            nc.sync.dma_start(eye_sb[:], eye_d[:])
            nc.sync.dma_start(mask_sb[:], mask_d[:])
            nc.sync.dma_start(wf_sb[:], wf_d.rearrange("(c p) n -> p c n", p=128))
            nc.sync.dma_start(bf_sb[:], bf_d[:])
            nc.sync.dma_start(ones_sb[:], ones_d[:])
            nc.gpsimd.memset(v_sb[:, :, 64:65], 1.0)

            # -- emission helpers ------------------------------------------
            def load_x(tc_i):
                tsl = slice(tc_i * 512, (tc_i + 1) * 512)
                xts = []
                for k in range(8):
                    xt = spx.tile([128, 512], F32R, tag="xt", name="xt")
                    nc.sync.dma_start(xt[:], xT_d[k * 128 : (k + 1) * 128, tsl])
                    xts.append(xt)
                return xts

            def proj_tasks(tc_i, xts):
                """Dense background tasks for chunk tc_i's projection+RoPE+vT."""
                tsl = slice(tc_i * 512, (tc_i + 1) * 512)

                def mk_group(mt):
                    def run():
                        pr = ps.tile([128, 512], F32, tag="tmp", name="tmp")
                        for k in range(8):
                            nc.tensor.matmul(
                                pr[:],
                                w_sb[:, k, mt * 128 : (mt + 1) * 128],
                                xts[k][:],
                                start=(k == 0),
                                stop=(k == 7),
                            )
                        nc.vector.tensor_scalar_add(
                            qkvT[mt][:, tsl], pr[:], bl_sb[:, mt : mt + 1]
                        )
                    return run

                def mk_rope_q(mt):
                    def run():
                        tmp = ps.tile([128, 512], F32, tag="tmp", name="tmp")
                        nc.tensor.matmul(
                            tmp[:], perm_sb[:], qkvT[mt][:, tsl], start=True, stop=True
                        )
                        nc.vector.tensor_mul(
                            qkvT[mt][:, tsl], qkvT[mt][:, tsl], cos_sb[:, tsl]
                        )
                        tmpb = spp.tile([128, 512], BF16, tag="p", name="p")
                        nc.vector.tensor_mul(tmpb[:], tmp[:], sin_sb[:, tsl])
                        nc.vector.tensor_add(qkvT[mt][:, tsl], qkvT[mt][:, tsl], tmpb[:])
                    return run

                def mk_vt(i):
                    def run():
                        tt = tc_i * 4 + i
                        vt = ps.tile([128, 512], BF16, tag="tmpb", name="tmpb", bufs=1)
                        nc.tensor.transpose(
                            vt[:, 0:64],
                            qkvT[2][64:128, tt * 128 : (tt + 1) * 128],
                            eye_sb[64:128, :],
                        )
                        nc.vector.tensor_copy(v_sb[:, tt, 0:64], vt[:, 0:64])
                    return run

                def rope_k():
                    tmp = ps.tile([128, 512], F32, tag="tmp", name="tmp")
                    nc.tensor.matmul(
                        tmp[0:64, :], perm_sb[:, 0:64], qkvT[2][:, tsl],
                        start=True, stop=True,
                    )
                    nc.vector.tensor_mul(
                        qkvT[2][0:64, tsl], qkvT[2][0:64, tsl], cos_sb[0:64, tsl]
                    )
                    tmpb = spp.tile([128, 512], BF16, tag="p", name="p")
                    nc.vector.tensor_mul(tmpb[0:64, :], tmp[0:64, :], sin_sb[0:64, tsl])
                    nc.vector.tensor_add(
                        qkvT[2][0:64, tsl], qkvT[2][0:64, tsl], tmpb[0:64, :]
                    )
                    nc.sync.dma_start(kdup[64:128, tsl], qkvT[2][0:64, tsl])

                return [
                    mk_group(0), mk_group(1), mk_group(2),
                    mk_rope_q(0), mk_rope_q(1),
                    mk_vt(0), mk_vt(1), mk_vt(2), mk_vt(3),
                    rope_k,
                ]

            def final_tasks(tc_i):
                tsl = slice(tc_i * 512, (tc_i + 1) * 512)

                def mk(nt):
                    def run():
                        y_ps = ps.tile([128, 512], F32, tag="tmp", name="tmp")
                        for cc in range(2):
                            nc.tensor.matmul(
                                y_ps[:],
                                wf_sb[:, cc, nt * 128 : (nt + 1) * 128],
                                oT_ab[cc][:, tsl],
                                start=(cc == 0),
                                stop=(cc == 1),
                            )
                        y_sb = spo.tile([128, 512], F32, tag="yout", name="yout")
                        nc.vector.tensor_scalar_add(
                            y_sb[:], y_ps[:], bf_sb[:, nt : nt + 1]
                        )
                        nc.sync.dma_start(yT_d[nt * 128 : (nt + 1) * 128, tsl], y_sb[:])
                    return run

                return [mk(nt) for nt in range(8)]

            # -- prologue --------------------------------------------------
            xts0 = load_x(0)
            for t in proj_tasks(0, xts0):
                t()

            # -- main loop -------------------------------------------------
            pending_norm = []  # deferred norm-finish closures

            def mk_norm(g, tci, o_ac, sums):
                tsl = slice(tci * 512, (tci + 1) * 512)
                odd = g % 2 == 1

                def run():
                    bc = ps.tile([128, 512], F32, tag="tmp", name="tmp")
                    nc.tensor.matmul(
                        bc[0:64, :], ones_sb[64:65, 0:64], sums[64:65, :],
                        start=True, stop=True,
                    )
                    bc_sb = spr.tile([64, 512], F32, tag="bcs", name="bcs")
                    nc.vector.reciprocal_approx_fast(out=bc_sb[:], in_=bc[0:64, :])
                    if odd:
                        stg = spr.tile([64, 512], F32R, tag="stg", name="stg")
                        nc.vector.tensor_mul(stg[:], o_ac[0:64, :], bc_sb[:])
                        nc.sync.dma_start(oT_ab[g // 2][64:128, tsl], stg[:])
                    else:
                        nc.vector.tensor_mul(
                            oT_ab[g // 2][0:64, tsl], o_ac[0:64, :], bc_sb[:]
                        )
                return run

            for tci in range(TCH):
                tsl = slice(tci * 512, (tci + 1) * 512)
                nblk = 4 * tci + 4

                bg = []
                if tci + 1 < TCH:
                    xts = load_x(tci + 1)
                    bg += proj_tasks(tci + 1, xts)
                if tci == 2:
                    bg += final_tasks(0)
                elif tci == 3:
                    bg += final_tasks(1) + final_tasks(2)
                bg_done = 0
                bg_total = len(bg)
                slots = G * nblk
                slot = 0

                for g in range(G):
                    odd = g % 2 == 1
                    qtile = qkvT[g // 2]
                    qrow = (g % 2) * 64
                    o_ac = psacc.tile([128, 512], F32, tag="oacc", name="oacc")
                    DEPTH = 3
                    pq = []

                    def emit_pv(jj, p_tile, o_ac=o_ac, nblk=nblk):
                        nc.tensor.matmul(
                            o_ac[0:65, :],
                            v_sb[:, jj, 0:65],
                            p_tile[:],
                            start=(jj == 0),
                            stop=(jj == nblk - 1),
                        )

                    for j in range(nblk):
                        s_ps = pss.tile([128, 512], F32, tag="s", name="s")
                        ksrc = (
                            kdup[64:128, j * 128 : (j + 1) * 128]
                            if odd
                            else qkvT[2][0:64, j * 128 : (j + 1) * 128]
                        )
                        nc.tensor.matmul(
                            s_ps[:], ksrc, qtile[qrow : qrow + 64, tsl],
                            start=True, stop=True,
                        )
                        p_sb = spp.tile([128, 512], BF16, tag="p", name="p")
                        nc.scalar.activation(p_sb[:], s_ps[:], AF.Exp)
                        if j >= 4 * tci:
                            off = 384 + 512 * tci - 128 * j
                            nc.gpsimd.tensor_mul(
                                p_sb[:], p_sb[:], mask_sb[:, off : off + 512]
                            )
                        pq.append((j, p_sb))
                        if len(pq) > DEPTH:
                            emit_pv(*pq.pop(0))
                        # drain deferred norms and background work
                        if pending_norm and slot % 2 == 1:
                            pending_norm.pop(0)()
                        slot += 1
                        due = bg_total * slot // slots
                        while bg_done < due:
                            bg[bg_done]()
                            bg_done += 1

                    for item in pq:
                        emit_pv(*item)
                    sums = spr.tile([128, 512], F32R, tag="rec", name="rec")
                    nc.vector.tensor_copy(sums[64:65, :], o_ac[64:65, :])
                    pending_norm.append(mk_norm(g, tci, o_ac, sums))

                while bg_done < bg_total:
                    bg[bg_done]()
                    bg_done += 1

            for fn in pending_norm:
                fn()
            for t in final_tasks(TCH - 1):
                t()

    nc.compile()
    return nc


def host_shard(inputs):
    """Build the 8 per-core input maps from full inputs."""
    x = np.ascontiguousarray(np.asarray(inputs["input"], dtype=np.float32))
    W = np.asarray(inputs["W_attn"], dtype=np.float32)
    bb = np.asarray(inputs["b_attn"], dtype=np.float32)
    Wf = np.asarray(inputs["W_final"], dtype=np.float32)
    bf = np.asarray(inputs["b_final"], dtype=np.float32)

    half = HS // 2
    inv_freq = MAX_PERIOD ** (-np.arange(half, dtype=np.float32) / half)
    ang = np.arange(T, dtype=np.float32)[:, None] * inv_freq  # (T, 32)
    sin_t = np.sin(ang).astype(np.float32)
    cos_t = np.cos(ang).astype(np.float32)
    cosT = np.repeat(cos_t.T, 2, axis=0)  # (64, T): row d -> cos(t*f[d//2])
    sgn = np.where(np.arange(HS) % 2 == 0, -1.0, 1.0).astype(np.float32)
    sinT = np.repeat(sin_t.T, 2, axis=0) * sgn[:, None]
    cos128 = np.ascontiguousarray(np.concatenate([cosT, cosT], axis=0))
    sin128 = np.ascontiguousarray(np.concatenate([sinT, sinT], axis=0))

    perm = np.zeros((128, 128), np.float32)
    idx = np.arange(128)
    perm[idx ^ 1, idx] = 1.0
    eye64 = np.zeros((128, 64), np.float32)
    eye64[64:128, :] = np.eye(64, dtype=np.float32)
    u = np.arange(896)
    maskb = (u[None, :] >= (np.arange(128)[:, None] + 384)).astype(np.float32)

    in_maps = []
    for cid in range(8):
        b, h = cid // 4, cid % 4
        qcols = np.concatenate(
            [np.arange(g * 256 + h * 64, g * 256 + h * 64 + 64) for g in range(G)]
        )
        kcols = np.arange(1024 + h * 64, 1024 + h * 64 + 64)
        vcols = np.arange(1280 + h * 64, 1280 + h * 64 + 64)
        cols = np.concatenate([qcols, kcols, vcols])
        w_loc = W[:, cols].copy()
        b_loc = bb[cols].copy()
        w_loc[:, :256] *= SCALE
        b_loc[:256] *= SCALE
        b_loc_m = np.ascontiguousarray(b_loc.reshape(3, 128).T)  # (128, 3)

        rows = np.concatenate(
            [np.arange(g * 256 + h * 64, g * 256 + h * 64 + 64) for g in range(G)]
        )
        wf_loc = np.ascontiguousarray(Wf[rows, :])  # (256, 1024)
        bf_m = (
            np.ascontiguousarray(bf.reshape(8, 128).T)
            if h == 0
            else np.zeros((128, 8), np.float32)
        )

        in_maps.append(
            {
                "xT": np.ascontiguousarray(x[b].T),
                "w_qkv": w_loc,
                "b_loc": b_loc_m,
                "cosT": cos128.astype(ml_dtypes.bfloat16),
                "sinT": sin128.astype(ml_dtypes.bfloat16),
                "perm": perm.astype(ml_dtypes.bfloat16),
                "eye64": eye64.astype(ml_dtypes.bfloat16),
                "maskb": maskb.astype(ml_dtypes.bfloat16),
                "wf": wf_loc,
                "bf": bf_m,
                "onesd": np.ones((128, 64), np.float32),
            }
        )
    return in_maps


def host_unshard(results):
    """Sum the 4 per-h partial yT per batch, transpose back to (B, T, C)."""
    out = np.empty((B, T, C), np.float32)
    for b in range(B):
        acc = results[b * 4]["yT"].astype(np.float32)
        for h in range(1, 4):
            acc = acc + results[b * 4 + h]["yT"]
        out[b] = acc.T
    return out


_NC_CACHE = None


def _get_nc():
    global _NC_CACHE
    if _NC_CACHE is None:
        _NC_CACHE = build_nc()
    return _NC_CACHE


def kernel(**inputs):
    nc = _get_nc()
    in_maps = host_shard(inputs)
    res = run_bass_kernel_spmd(nc, in_maps, core_ids=list(range(8)))
    return host_unshard(res.results)


# revision 26
# speedup vs baseline: 2.8726x; 1.1294x over previous
"""Trainium2 Bass kernel for grouped-query causal attention (B=2, T=2048, C=1024,
16 q heads / 4 kv heads, RoPE, fused qkv + output projection).

Sharding: 8 cores = (batch b, kv-head h). Each core:
  - projects x -> qT (4 heads), kT, vT with pre-sliced/pre-scaled weights
    (transposed layout: channels on partitions, T on free dim)
  - applies RoPE (pair-swap via permutation matmul on PE + DVE mul/add)
  - causal attention for its 4 query heads (S^T blocks, exp without
    max-subtraction [logits are O(8)], softmax denominators via a ones
    column appended to V, post-exp 0/1 causal mask)
  - partial output projection y^T = Wf_local^T @ oT  (transposed)
Host sums the 4 per-h partials per batch and transposes back.
"""

import sys

sys.path.insert(0, "/opt/trn_rl_repo")

import ml_dtypes
import numpy as np

import concourse.bacc as bacc
import concourse.mybir as mybir
from concourse import tile
from concourse.bass_utils import run_bass_kernel_spmd

B, T, C = 2, 2048, 1024
G, HKV, HS = 4, 4, 64
OUT_DIM = C + 2 * (C // G)
SCALE = 1.0 / np.sqrt(HS)
MAX_PERIOD = 10000.0

F32 = mybir.dt.float32
F32R = mybir.dt.float32r
BF16 = mybir.dt.bfloat16
AF = mybir.ActivationFunctionType


TCH = T // 512  # 4 chunks of 512 along T
NT = T // 128  # 16 tiles of 128 along T


def build_nc():
    nc = bacc.Bacc(None, target_bir_lowering=False)

    xT_d = nc.dram_tensor("xT", [C, T], F32R, kind="ExternalInput")
    w_d = nc.dram_tensor("w_qkv", [C, 384], F32R, kind="ExternalInput")
    bl_d = nc.dram_tensor("b_loc", [128, 3], F32, kind="ExternalInput")
    cos_d = nc.dram_tensor("cosT", [128, T], BF16, kind="ExternalInput")
    sin_d = nc.dram_tensor("sinT", [128, T], BF16, kind="ExternalInput")
    perm_d = nc.dram_tensor("perm", [128, 128], BF16, kind="ExternalInput")
    eye_d = nc.dram_tensor("eye64", [128, 64], BF16, kind="ExternalInput")
    mask_d = nc.dram_tensor("maskb", [128, 2, 896], BF16, kind="ExternalInput")
    wf_d = nc.dram_tensor("wf", [256, 1024], F32R, kind="ExternalInput")
    bf_d = nc.dram_tensor("bf", [128, 8], F32, kind="ExternalInput")
    ones_d = nc.dram_tensor("onesd", [128, 64], F32R, kind="ExternalInput")
    yT_d = nc.dram_tensor("yT", [C, T], F32, kind="ExternalOutput")

    with tile.TileContext(nc) as tc:
        with (
            tc.tile_pool(name="persist", bufs=1) as pp,
            tc.tile_pool(name="xstream", bufs=10) as spx,
            tc.tile_pool(name="pstream", bufs=19) as spp,
            tc.tile_pool(name="rstream", bufs=3) as spr,
            tc.tile_pool(name="ostream", bufs=3) as spo,
            tc.tile_pool(name="ps_acc", bufs=3, space="PSUM") as psacc,
            tc.tile_pool(name="ps_s", bufs=2, space="PSUM") as pss,
            tc.tile_pool(name="ps_tmp", bufs=1, space="PSUM") as ps,
        ):
            # ---- persistent tiles ----
            w_sb = pp.tile([128, 8, 384], F32R, tag="w", name="w")
            bl_sb = pp.tile([128, 3], F32, tag="bl", name="bl")
            cos_sb = pp.tile([128, T], BF16, tag="cos", name="cos")
            sin_sb = pp.tile([128, T], BF16, tag="sin", name="sin")
            perm_sb = pp.tile([128, 128], BF16, tag="perm", name="perm")
            eye_sb = pp.tile([128, 64], BF16, tag="eye", name="eye")
            mask_sb = pp.tile([128, 2, 896], BF16, tag="mask", name="mask")
            wf_sb = pp.tile([128, 2, 1024], F32R, tag="wf", name="wf")
            bf_sb = pp.tile([128, 8], F32, tag="bf", name="bf")
            ones_sb = pp.tile([128, 64], F32R, tag="ones", name="ones")
            qkvT = [pp.tile([128, T], BF16, tag=f"qkvT{m}", name=f"qkvT{m}") for m in range(3)]
            qcat = [pp.tile([64, 2, T], BF16, tag=f"qcat{m}", name=f"qcat{m}") for m in range(2)]
            v_sb = pp.tile([128, NT, 65], BF16, tag="vaug", name="vaug")
            oT_ab = [pp.tile([128, T], F32R, tag=f"oT{i}", name=f"oT{i}") for i in range(2)]

            for k in range(8):
                nc.sync.dma_start(
                    w_sb[:, k, :], w_d[k * 128 : (k + 1) * 128, :]
                )
            nc.sync.dma_start(bl_sb[:], bl_d[:])
            nc.sync.dma_start(cos_sb[:], cos_d[:])
            nc.sync.dma_start(sin_sb[:], sin_d[:])
            nc.sync.dma_start(perm_sb[:], perm_d[:])
            nc.sync.dma_start(eye_sb[:], eye_d[:])
            nc.sync.dma_start(mask_sb[:], mask_d[:])
            nc.sync.dma_start(wf_sb[:], wf_d.rearrange("(c p) n -> p c n", p=128))
            nc.sync.dma_start(bf_sb[:], bf_d[:])
            nc.sync.dma_start(ones_sb[:], ones_d[:])
            nc.gpsimd.memset(v_sb[:, :, 64:65], 1.0)

            # -- emission helpers ------------------------------------------
            def load_x(tc_i):
                tsl = slice(tc_i * 512, (tc_i + 1) * 512)
                xts = []
                for k in range(8):
                    xt = spx.tile([128, 512], F32R, tag="xt", name="xt")
                    nc.sync.dma_start(xt[:], xT_d[k * 128 : (k + 1) * 128, tsl])
                    xts.append(xt)
                return xts

            def proj_tasks(tc_i, xts):
                """Dense background tasks for chunk tc_i's projection+RoPE+vT."""
                tsl = slice(tc_i * 512, (tc_i + 1) * 512)

                def mk_group(mt):
                    def run():
                        pr = ps.tile([128, 512], F32, tag="tmp", name="tmp")
                        for k in range(8):
                            nc.tensor.matmul(
                                pr[:],
                                w_sb[:, k, mt * 128 : (mt + 1) * 128],
                                xts[k][:],
                                start=(k == 0),
                                stop=(k == 7),
                            )
                        nc.vector.tensor_scalar_add(
                            qkvT[mt][:, tsl], pr[:], bl_sb[:, mt : mt + 1]
                        )
                    return run

                def mk_rope_q(mt):
                    def run():
                        tmp = ps.tile([128, 512], F32, tag="tmp", name="tmp")
                        nc.tensor.matmul(
                            tmp[:], perm_sb[:], qkvT[mt][:, tsl], start=True, stop=True
                        )
                        nc.vector.tensor_mul(
                            qkvT[mt][:, tsl], qkvT[mt][:, tsl], cos_sb[:, tsl]
                        )
                        tmpb = spp.tile([128, 1024], BF16, tag="p", name="p")
                        nc.vector.tensor_mul(tmpb[:, 0:512], tmp[:], sin_sb[:, tsl])
                        nc.vector.tensor_add(
                            qkvT[mt][:, tsl], qkvT[mt][:, tsl], tmpb[:, 0:512]
                        )
                        nc.sync.dma_start(qcat[mt][:, 0, tsl], qkvT[mt][0:64, tsl])
                        nc.sync.dma_start(qcat[mt][:, 1, tsl], qkvT[mt][64:128, tsl])
                    return run

                def mk_vt(i):
                    def run():
                        tt = tc_i * 4 + i
                        vt = ps.tile([128, 512], BF16, tag="tmp", name="tmp")
                        nc.tensor.transpose(
                            vt[:, 0:64],
                            qkvT[2][64:128, tt * 128 : (tt + 1) * 128],
                            eye_sb[64:128, :],
                        )
                        nc.vector.tensor_copy(v_sb[:, tt, 0:64], vt[:, 0:64])
                    return run

                def rope_k():
                    tmp = ps.tile([128, 512], F32, tag="tmp", name="tmp")
                    nc.tensor.matmul(
                        tmp[0:64, :], perm_sb[:, 0:64], qkvT[2][:, tsl],
                        start=True, stop=True,
                    )
                    nc.vector.tensor_mul(
                        qkvT[2][0:64, tsl], qkvT[2][0:64, tsl], cos_sb[0:64, tsl]
                    )
                    tmpb = spp.tile([128, 1024], BF16, tag="p", name="p")
                    nc.vector.tensor_mul(
                        tmpb[0:64, 0:512], tmp[0:64, :], sin_sb[0:64, tsl]
                    )
                    nc.vector.tensor_add(
                        qkvT[2][0:64, tsl], qkvT[2][0:64, tsl], tmpb[0:64, 0:512]
                    )

                return [
                    mk_group(0), mk_group(1), mk_group(2),
                    mk_rope_q(0), mk_rope_q(1),
                    mk_vt(0), mk_vt(1), mk_vt(2), mk_vt(3),
                    rope_k,
                ]

            def final_tasks(tc_i):
                tsl = slice(tc_i * 512, (tc_i + 1) * 512)

                def mk(nt):
                    def run():
                        y_ps = ps.tile([128, 512], F32, tag="tmp", name="tmp")
                        for cc in range(2):
                            nc.tensor.matmul(
                                y_ps[:],
                                wf_sb[:, cc, nt * 128 : (nt + 1) * 128],
                                oT_ab[cc][:, tsl],
                                start=(cc == 0),
                                stop=(cc == 1),
                            )
                        y_sb = spo.tile([128, 512], F32, tag="yout", name="yout")
                        nc.vector.tensor_scalar_add(
                            y_sb[:], y_ps[:], bf_sb[:, nt : nt + 1]
                        )
                        nc.sync.dma_start(yT_d[nt * 128 : (nt + 1) * 128, tsl], y_sb[:])
                    return run

                return [mk(nt) for nt in range(8)]

            # -- prologue --------------------------------------------------
            xts0 = load_x(0)
            for t in proj_tasks(0, xts0):
                t()

            # -- main loop -------------------------------------------------
            pending_norm = []  # deferred norm-finish closures

            def mk_norm(g, tci, o_ac, sums):
                tsl = slice(tci * 512, (tci + 1) * 512)
                odd = g % 2 == 1

                def run():
                    bc = ps.tile([128, 512], F32, tag="tmp", name="tmp")
                    nc.tensor.matmul(
                        bc[0:64, :], ones_sb[64:65, 0:64], sums[64:65, :],
                        start=True, stop=True,
                    )
                    bc_sb = spr.tile([64, 512], F32, tag="bcs", name="bcs")
                    nc.vector.reciprocal_approx_fast(out=bc_sb[:], in_=bc[0:64, :])
                    if odd:
                        stg = spr.tile([64, 512], F32R, tag="stg", name="stg")
                        nc.vector.tensor_mul(stg[:], o_ac[0:64, :], bc_sb[:])
                        nc.sync.dma_start(oT_ab[g // 2][64:128, tsl], stg[:])
                    else:
                        nc.vector.tensor_mul(
                            oT_ab[g // 2][0:64, tsl], o_ac[0:64, :], bc_sb[:]
                        )
                return run

            for tci in range(TCH):
                tsl = slice(tci * 512, (tci + 1) * 512)
                nblk = 4 * tci + 4

                bg = []
                if tci + 1 < TCH:
                    xts = load_x(tci + 1)
                    bg += proj_tasks(tci + 1, xts)
                if tci == 2:
                    bg += final_tasks(0)
                elif tci == 3:
                    bg += final_tasks(1) + final_tasks(2)
                bg_done = 0
                bg_total = len(bg)
                slots = G * nblk
                slot = 0

                for pair in range(2):
                    qc = qcat[pair]
                    o_acs = [
                        psacc.tile([128, 512], F32, tag="oacc", name="oacc")
                        for _ in range(2)
                    ]
                    DEPTH = 2
                    pq = []       # (j, p_view) waiting for h0 PV
                    plist = []    # all (j, p_view) for h1's dense tail

                    def emit_pv(jj, h01, p_tile, o_acs=o_acs, nblk=nblk):
                        nc.tensor.matmul(
                            o_acs[h01][0:65, :],
                            v_sb[:, jj, 0:65],
                            p_tile[:, h01, :],
                            start=(jj == 0),
                            stop=(jj == nblk - 1),
                        )

                    for j in range(nblk):
                        s_ps = pss.tile([128, 1024], F32, tag="s", name="s")
                        for h01 in range(2):
                            nc.tensor.matmul(
                                s_ps[:, h01 * 512 : (h01 + 1) * 512],
                                qkvT[2][0:64, j * 128 : (j + 1) * 128],
                                qc[:, h01, tsl],
                                start=True,
                                stop=True,
                            )
                        p_sb = spp.tile([128, 1024], BF16, tag="p", name="p")
                        nc.scalar.activation(p_sb[:], s_ps[:], AF.Exp)
                        pp_view = p_sb[:].rearrange("q (h t) -> q h t", h=2)
                        if j >= 4 * tci:
                            off = 384 + 512 * tci - 128 * j
                            nc.gpsimd.tensor_mul(
                                pp_view,
                                pp_view,
                                mask_sb[:, :, off : off + 512],
                            )
                        pq.append((j, pp_view))
                        plist.append((j, pp_view))
                        if len(pq) > DEPTH:
                            jj, pv = pq.pop(0)
                            emit_pv(jj, 0, pv)
                        if pending_norm and j % 2 == 1:
                            pending_norm.pop(0)()
                        slot += 2
                        due = bg_total * min(slot, slots) // slots
                        while bg_done < due:
                            bg[bg_done]()
                            bg_done += 1

                    for jj, pv in pq:
                        emit_pv(jj, 0, pv)
                    # head 1: dense back-to-back PV run (single open group)
                    for jj, pv in plist:
                        emit_pv(jj, 1, pv)
                    for h01 in range(2):
                        g = pair * 2 + h01
                        o_ac = o_acs[h01]
                        sums = spr.tile([128, 512], F32R, tag="rec", name="rec")
                        nc.vector.tensor_copy(sums[64:65, :], o_ac[64:65, :])
                        pending_norm.append(mk_norm(g, tci, o_ac, sums))

                while bg_done < bg_total:
                    bg[bg_done]()
                    bg_done += 1

            for fn in pending_norm:
                fn()
            for t in final_tasks(TCH - 1):
                t()

    nc.compile()
    return nc


def host_shard(inputs):
    """Build the 8 per-core input maps from full inputs."""
    x = np.ascontiguousarray(np.asarray(inputs["input"], dtype=np.float32))
    W = np.asarray(inputs["W_attn"], dtype=np.float32)
    bb = np.asarray(inputs["b_attn"], dtype=np.float32)
    Wf = np.asarray(inputs["W_final"], dtype=np.float32)
    bf = np.asarray(inputs["b_final"], dtype=np.float32)

    half = HS // 2
    inv_freq = MAX_PERIOD ** (-np.arange(half, dtype=np.float32) / half)
    ang = np.arange(T, dtype=np.float32)[:, None] * inv_freq  # (T, 32)
    sin_t = np.sin(ang).astype(np.float32)
    cos_t = np.cos(ang).astype(np.float32)
    cosT = np.repeat(cos_t.T, 2, axis=0)  # (64, T): row d -> cos(t*f[d//2])
    sgn = np.where(np.arange(HS) % 2 == 0, -1.0, 1.0).astype(np.float32)
    sinT = np.repeat(sin_t.T, 2, axis=0) * sgn[:, None]
    cos128 = np.ascontiguousarray(np.concatenate([cosT, cosT], axis=0))
    sin128 = np.ascontiguousarray(np.concatenate([sinT, sinT], axis=0))

    perm = np.zeros((128, 128), np.float32)
    idx = np.arange(128)
    perm[idx ^ 1, idx] = 1.0
    eye64 = np.zeros((128, 64), np.float32)
    eye64[64:128, :] = np.eye(64, dtype=np.float32)
    u = np.arange(896)
    mb = (u[None, :] >= (np.arange(128)[:, None] + 384)).astype(np.float32)
    maskb = np.ascontiguousarray(np.stack([mb, mb], axis=1))  # (128, 2, 896)

    in_maps = []
    for cid in range(8):
        b, h = cid // 4, cid % 4
        qcols = np.concatenate(
            [np.arange(g * 256 + h * 64, g * 256 + h * 64 + 64) for g in range(G)]
        )
        kcols = np.arange(1024 + h * 64, 1024 + h * 64 + 64)
        vcols = np.arange(1280 + h * 64, 1280 + h * 64 + 64)
        cols = np.concatenate([qcols, kcols, vcols])
        w_loc = W[:, cols].copy()
        b_loc = bb[cols].copy()
        w_loc[:, :256] *= SCALE
        b_loc[:256] *= SCALE
        b_loc_m = np.ascontiguousarray(b_loc.reshape(3, 128).T)  # (128, 3)

        rows = np.concatenate(
            [np.arange(g * 256 + h * 64, g * 256 + h * 64 + 64) for g in range(G)]
        )
        wf_loc = np.ascontiguousarray(Wf[rows, :])  # (256, 1024)
        bf_m = (
            np.ascontiguousarray(bf.reshape(8, 128).T)
            if h == 0
            else np.zeros((128, 8), np.float32)
        )

        in_maps.append(
            {
                "xT": np.ascontiguousarray(x[b].T),
                "w_qkv": w_loc,
                "b_loc": b_loc_m,
                "cosT": cos128.astype(ml_dtypes.bfloat16),
                "sinT": sin128.astype(ml_dtypes.bfloat16),
                "perm": perm.astype(ml_dtypes.bfloat16),
                "eye64": eye64.astype(ml_dtypes.bfloat16),
                "maskb": maskb.astype(ml_dtypes.bfloat16),
                "wf": wf_loc,
                "bf": bf_m,
                "onesd": np.ones((128, 64), np.float32),
            }
        )
    return in_maps


def host_unshard(results):
    """Sum the 4 per-h partial yT per batch, transpose back to (B, T, C)."""
    out = np.empty((B, T, C), np.float32)
    for b in range(B):
        acc = results[b * 4]["yT"].astype(np.float32)
        for h in range(1, 4):
            acc = acc + results[b * 4 + h]["yT"]
        out[b] = acc.T
    return out


_NC_CACHE = None


def _get_nc():
    global _NC_CACHE
    if _NC_CACHE is None:
        _NC_CACHE = build_nc()
    return _NC_CACHE


def kernel(**inputs):
    nc = _get_nc()
    in_maps = host_shard(inputs)
    res = run_bass_kernel_spmd(nc, in_maps, core_ids=list(range(8)))
    return host_unshard(res.results)


# revision 28
# speedup vs baseline: 3.0612x; 1.0656x over previous
"""Trainium2 Bass kernel for grouped-query causal attention (B=2, T=2048, C=1024,
16 q heads / 4 kv heads, RoPE, fused qkv + output projection).

Sharding: 8 cores = (batch b, kv-head h). Each core:
  - projects x -> qT (4 heads), kT, vT with pre-sliced/pre-scaled weights
    (transposed layout: channels on partitions, T on free dim)
  - applies RoPE (pair-swap via permutation matmul on PE + DVE mul/add)
  - causal attention for its 4 query heads (S^T blocks, exp without
    max-subtraction [logits are O(8)], softmax denominators via a ones
    column appended to V, post-exp 0/1 causal mask)
  - partial output projection y^T = Wf_local^T @ oT  (transposed)
Host sums the 4 per-h partials per batch and transposes back.
"""

import sys

sys.path.insert(0, "/opt/trn_rl_repo")

import ml_dtypes
import numpy as np

import concourse.bacc as bacc
import concourse.mybir as mybir
from concourse import tile
from concourse.bass_utils import run_bass_kernel_spmd

B, T, C = 2, 2048, 1024
G, HKV, HS = 4, 4, 64
OUT_DIM = C + 2 * (C // G)
SCALE = 1.0 / np.sqrt(HS)
MAX_PERIOD = 10000.0

F32 = mybir.dt.float32
F32R = mybir.dt.float32r
BF16 = mybir.dt.bfloat16
AF = mybir.ActivationFunctionType


TCH = T // 512  # 4 chunks of 512 along T
NT = T // 128  # 16 tiles of 128 along T


def build_nc():
    nc = bacc.Bacc(None, target_bir_lowering=False)

    xT_d = nc.dram_tensor("xT", [C, T], F32R, kind="ExternalInput")
    w_d = nc.dram_tensor("w_qkv", [C, 384], F32R, kind="ExternalInput")
    bl_d = nc.dram_tensor("b_loc", [128, 3], F32, kind="ExternalInput")
    cos_d = nc.dram_tensor("cosT", [128, T], BF16, kind="ExternalInput")
    sin_d = nc.dram_tensor("sinT", [128, T], BF16, kind="ExternalInput")
    perm_d = nc.dram_tensor("perm", [128, 128], BF16, kind="ExternalInput")
    eye_d = nc.dram_tensor("eye64", [128, 64], BF16, kind="ExternalInput")
    mask_d = nc.dram_tensor("maskb", [128, 2, 896], BF16, kind="ExternalInput")
    wf_d = nc.dram_tensor("wf", [256, 1024], F32R, kind="ExternalInput")
    bf_d = nc.dram_tensor("bf", [128, 8], F32, kind="ExternalInput")
    ones_d = nc.dram_tensor("onesd", [128, 64], F32R, kind="ExternalInput")
    yT_d = nc.dram_tensor("yT", [C, T], F32, kind="ExternalOutput")

    with tile.TileContext(nc) as tc:
        with (
            tc.tile_pool(name="persist", bufs=1) as pp,
            tc.tile_pool(name="xstream", bufs=10) as spx,
            tc.tile_pool(name="pstream", bufs=19) as spp,
            tc.tile_pool(name="rstream", bufs=3) as spr,
            tc.tile_pool(name="ostream", bufs=3) as spo,
            tc.tile_pool(name="ps_acc", bufs=3, space="PSUM") as psacc,
            tc.tile_pool(name="ps_s", bufs=2, space="PSUM") as pss,
            tc.tile_pool(name="ps_tmp", bufs=1, space="PSUM") as ps,
        ):
            # ---- persistent tiles ----
            w_sb = pp.tile([128, 8, 384], F32R, tag="w", name="w")
            bl_sb = pp.tile([128, 3], F32, tag="bl", name="bl")
            cos_sb = pp.tile([128, T], BF16, tag="cos", name="cos")
            sin_sb = pp.tile([128, T], BF16, tag="sin", name="sin")
            perm_sb = pp.tile([128, 128], BF16, tag="perm", name="perm")
            eye_sb = pp.tile([128, 64], BF16, tag="eye", name="eye")
            mask_sb = pp.tile([128, 2, 896], BF16, tag="mask", name="mask")
            wf_sb = pp.tile([128, 2, 1024], F32R, tag="wf", name="wf")
            bf_sb = pp.tile([128, 8], F32, tag="bf", name="bf")
            ones_sb = pp.tile([128, 64], F32R, tag="ones", name="ones")
            qkvT = [pp.tile([128, T], BF16, tag=f"qkvT{m}", name=f"qkvT{m}") for m in range(3)]
            qcat = [pp.tile([64, 2, T], BF16, tag=f"qcat{m}", name=f"qcat{m}") for m in range(2)]
            v_sb = pp.tile([128, NT, 65], BF16, tag="vaug", name="vaug")
            oT_ab = [pp.tile([128, T], F32R, tag=f"oT{i}", name=f"oT{i}") for i in range(2)]

            nc.sync.dma_start(bl_sb[:], bl_d[:])
            nc.gpsimd.memset(v_sb[:, :, 64:65], 1.0)

            # -- emission helpers ------------------------------------------
            def load_x(tc_i):
                tsl = slice(tc_i * 512, (tc_i + 1) * 512)
                xts = []
                for k in range(8):
                    xt = spx.tile([128, 512], F32R, tag="xt", name="xt")
                    nc.sync.dma_start(xt[:], xT_d[k * 128 : (k + 1) * 128, tsl])
                    xts.append(xt)
                return xts

            def proj_tasks(tc_i, xts):
                """Dense background tasks for chunk tc_i's projection+RoPE+vT."""
                tsl = slice(tc_i * 512, (tc_i + 1) * 512)

                def mk_group(mt):
                    def run():
                        pr = ps.tile([128, 512], F32, tag="tmp", name="tmp")
                        for k in range(8):
                            nc.tensor.matmul(
                                pr[:],
                                w_sb[:, k, mt * 128 : (mt + 1) * 128],
                                xts[k][:],
                                start=(k == 0),
                                stop=(k == 7),
                            )
                        nc.vector.tensor_scalar_add(
                            qkvT[mt][:, tsl], pr[:], bl_sb[:, mt : mt + 1]
                        )
                    return run

                def mk_rope_q(mt):
                    def run():
                        tmp = ps.tile([128, 512], F32, tag="tmp", name="tmp")
                        nc.tensor.matmul(
                            tmp[:], perm_sb[:], qkvT[mt][:, tsl], start=True, stop=True
                        )
                        nc.vector.tensor_mul(
                            qkvT[mt][:, tsl], qkvT[mt][:, tsl], cos_sb[:, tsl]
                        )
                        tmpb = spp.tile([128, 1024], BF16, tag="p", name="p")
                        nc.vector.tensor_mul(tmpb[:, 0:512], tmp[:], sin_sb[:, tsl])
                        nc.vector.tensor_add(
                            qkvT[mt][:, tsl], qkvT[mt][:, tsl], tmpb[:, 0:512]
                        )
                        nc.sync.dma_start(qcat[mt][:, 0, tsl], qkvT[mt][0:64, tsl])
                        nc.sync.dma_start(qcat[mt][:, 1, tsl], qkvT[mt][64:128, tsl])
                    return run

                def mk_vt(i):
                    def run():
                        tt = tc_i * 4 + i
                        vt = ps.tile([128, 512], BF16, tag="tmp", name="tmp")
                        nc.tensor.transpose(
                            vt[:, 0:64],
                            qkvT[2][64:128, tt * 128 : (tt + 1) * 128],
                            eye_sb[64:128, :],
                        )
                        nc.vector.tensor_copy(v_sb[:, tt, 0:64], vt[:, 0:64])
                    return run

                def rope_k():
                    tmp = ps.tile([128, 512], F32, tag="tmp", name="tmp")
                    nc.tensor.matmul(
                        tmp[0:64, :], perm_sb[:, 0:64], qkvT[2][:, tsl],
                        start=True, stop=True,
                    )
                    nc.vector.tensor_mul(
                        qkvT[2][0:64, tsl], qkvT[2][0:64, tsl], cos_sb[0:64, tsl]
                    )
                    tmpb = spp.tile([128, 1024], BF16, tag="p", name="p")
                    nc.vector.tensor_mul(
                        tmpb[0:64, 0:512], tmp[0:64, :], sin_sb[0:64, tsl]
                    )
                    nc.vector.tensor_add(
                        qkvT[2][0:64, tsl], qkvT[2][0:64, tsl], tmpb[0:64, 0:512]
                    )

                return [
                    mk_group(0), mk_group(1), mk_group(2),
                    mk_rope_q(0), mk_rope_q(1),
                    mk_vt(0), mk_vt(1), mk_vt(2), mk_vt(3),
                    rope_k,
                ]

            def final_tasks(tc_i):
                tsl = slice(tc_i * 512, (tc_i + 1) * 512)

                def mk(nt):
                    def run():
                        y_ps = ps.tile([128, 512], F32, tag="tmp", name="tmp")
                        for cc in range(2):
                            nc.tensor.matmul(
                                y_ps[:],
                                wf_sb[:, cc, nt * 128 : (nt + 1) * 128],
                                oT_ab[cc][:, tsl],
                                start=(cc == 0),
                                stop=(cc == 1),
                            )
                        y_sb = spo.tile([128, 512], F32, tag="yout", name="yout")
                        nc.vector.tensor_scalar_add(
                            y_sb[:], y_ps[:], bf_sb[:, nt : nt + 1]
                        )
                        nc.sync.dma_start(yT_d[nt * 128 : (nt + 1) * 128, tsl], y_sb[:])
                    return run

                return [mk(nt) for nt in range(8)]

            # -- prologue --------------------------------------------------
            xts0 = load_x(0)
            for k in range(8):
                nc.sync.dma_start(
                    w_sb[:, k, :], w_d[k * 128 : (k + 1) * 128, :]
                )
            nc.sync.dma_start(perm_sb[:], perm_d[:])
            nc.sync.dma_start(cos_sb[:], cos_d[:])
            nc.sync.dma_start(sin_sb[:], sin_d[:])
            nc.sync.dma_start(eye_sb[:], eye_d[:])
            nc.sync.dma_start(mask_sb[:], mask_d[:])
            nc.sync.dma_start(wf_sb[:], wf_d.rearrange("(c p) n -> p c n", p=128))
            nc.sync.dma_start(bf_sb[:], bf_d[:])
            nc.sync.dma_start(ones_sb[:], ones_d[:])
            for t in proj_tasks(0, xts0):
                t()

            # -- main loop -------------------------------------------------
            pending_norm = []  # deferred norm-finish closures

            def mk_norm(g, tci, o_ac, sums):
                tsl = slice(tci * 512, (tci + 1) * 512)
                odd = g % 2 == 1

                def run():
                    bc = ps.tile([128, 512], F32, tag="tmp", name="tmp")
                    nc.tensor.matmul(
                        bc[0:64, :], ones_sb[64:65, 0:64], sums[64:65, :],
                        start=True, stop=True,
                    )
                    bc_sb = spr.tile([64, 512], F32, tag="bcs", name="bcs")
                    nc.vector.reciprocal_approx_fast(out=bc_sb[:], in_=bc[0:64, :])
                    if odd:
                        stg = spr.tile([64, 512], F32R, tag="stg", name="stg")
                        nc.vector.tensor_mul(stg[:], o_ac[0:64, :], bc_sb[:])
                        nc.sync.dma_start(oT_ab[g // 2][64:128, tsl], stg[:])
                    else:
                        nc.vector.tensor_mul(
                            oT_ab[g // 2][0:64, tsl], o_ac[0:64, :], bc_sb[:]
                        )
                return run

            for tci in range(TCH):
                tsl = slice(tci * 512, (tci + 1) * 512)
                nblk = 4 * tci + 4

                bg = []
                if tci + 1 < TCH:
                    xts = load_x(tci + 1)
                    bg += proj_tasks(tci + 1, xts)
                if tci == 2:
                    bg += final_tasks(0)
                elif tci == 3:
                    bg += final_tasks(1) + final_tasks(2)
                bg_done = 0
                bg_total = len(bg)
                slots = G * nblk
                slot = 0

                for pair in range(2):
                    qc = qcat[pair]
                    o_acs = [
                        psacc.tile([128, 512], F32, tag="oacc", name="oacc")
                        for _ in range(2)
                    ]
                    DEPTH = 2
                    pq = []       # (j, p_view) waiting for h0 PV
                    plist = []    # all (j, p_view) for h1's dense tail

                    def emit_pv(jj, h01, p_tile, o_acs=o_acs, nblk=nblk):
                        nc.tensor.matmul(
                            o_acs[h01][0:65, :],
                            v_sb[:, jj, 0:65],
                            p_tile[:, h01, :],
                            start=(jj == 0),
                            stop=(jj == nblk - 1),
                        )

                    for j in range(nblk):
                        s_ps = pss.tile([128, 1024], F32, tag="s", name="s")
                        for h01 in range(2):
                            nc.tensor.matmul(
                                s_ps[:, h01 * 512 : (h01 + 1) * 512],
                                qkvT[2][0:64, j * 128 : (j + 1) * 128],
                                qc[:, h01, tsl],
                                start=True,
                                stop=True,
                            )
                        p_sb = spp.tile([128, 1024], BF16, tag="p", name="p")
                        nc.scalar.activation(p_sb[:], s_ps[:], AF.Exp)
                        pp_view = p_sb[:].rearrange("q (h t) -> q h t", h=2)
                        if j >= 4 * tci:
                            off = 384 + 512 * tci - 128 * j
                            nc.gpsimd.tensor_mul(
                                pp_view,
                                pp_view,
                                mask_sb[:, :, off : off + 512],
                            )
                        pq.append((j, pp_view))
                        plist.append((j, pp_view))
                        if len(pq) > DEPTH:
                            jj, pv = pq.pop(0)
                            emit_pv(jj, 0, pv)
                        if pending_norm and j % 2 == 1:
                            pending_norm.pop(0)()
                        slot += 2
                        due = bg_total * min(slot, slots) // slots
                        while bg_done < due:
                            bg[bg_done]()
                            bg_done += 1

                    for jj, pv in pq:
                        emit_pv(jj, 0, pv)
                    # head 1: dense back-to-back PV run (single open group)
                    for jj, pv in plist:
                        emit_pv(jj, 1, pv)
                    for h01 in range(2):
                        g = pair * 2 + h01
                        o_ac = o_acs[h01]
                        sums = spr.tile([128, 512], F32R, tag="rec", name="rec")
                        nc.vector.tensor_copy(sums[64:65, :], o_ac[64:65, :])
                        pending_norm.append(mk_norm(g, tci, o_ac, sums))

                while bg_done < bg_total:
                    bg[bg_done]()
                    bg_done += 1

            for fn in pending_norm:
                fn()
            for t in final_tasks(TCH - 1):
                t()

    nc.compile()
    return nc


def host_shard(inputs):
    """Build the 8 per-core input maps from full inputs."""
    x = np.ascontiguousarray(np.asarray(inputs["input"], dtype=np.float32))
    W = np.asarray(inputs["W_attn"], dtype=np.float32)
    bb = np.asarray(inputs["b_attn"], dtype=np.float32)
    Wf = np.asarray(inputs["W_final"], dtype=np.float32)
    bf = np.asarray(inputs["b_final"], dtype=np.float32)

    half = HS // 2
    inv_freq = MAX_PERIOD ** (-np.arange(half, dtype=np.float32) / half)
    ang = np.arange(T, dtype=np.float32)[:, None] * inv_freq  # (T, 32)
    sin_t = np.sin(ang).astype(np.float32)
    cos_t = np.cos(ang).astype(np.float32)
    cosT = np.repeat(cos_t.T, 2, axis=0)  # (64, T): row d -> cos(t*f[d//2])
    sgn = np.where(np.arange(HS) % 2 == 0, -1.0, 1.0).astype(np.float32)
    sinT = np.repeat(sin_t.T, 2, axis=0) * sgn[:, None]
    cos128 = np.ascontiguousarray(np.concatenate([cosT, cosT], axis=0))
    sin128 = np.ascontiguousarray(np.concatenate([sinT, sinT], axis=0))

    perm = np.zeros((128, 128), np.float32)
    idx = np.arange(128)
    perm[idx ^ 1, idx] = 1.0
    eye64 = np.zeros((128, 64), np.float32)
    eye64[64:128, :] = np.eye(64, dtype=np.float32)
    u = np.arange(896)
    mb = (u[None, :] >= (np.arange(128)[:, None] + 384)).astype(np.float32)
    maskb = np.ascontiguousarray(np.stack([mb, mb], axis=1))  # (128, 2, 896)

    in_maps = []
    for cid in range(8):
        b, h = cid // 4, cid % 4
        qcols = np.concatenate(
            [np.arange(g * 256 + h * 64, g * 256 + h * 64 + 64) for g in range(G)]
        )
        kcols = np.arange(1024 + h * 64, 1024 + h * 64 + 64)
        vcols = np.arange(1280 + h * 64, 1280 + h * 64 + 64)
        cols = np.concatenate([qcols, kcols, vcols])
        w_loc = W[:, cols].copy()
        b_loc = bb[cols].copy()
        w_loc[:, :256] *= SCALE
        b_loc[:256] *= SCALE
        b_loc_m = np.ascontiguousarray(b_loc.reshape(3, 128).T)  # (128, 3)

        rows = np.concatenate(
            [np.arange(g * 256 + h * 64, g * 256 + h * 64 + 64) for g in range(G)]
        )
        wf_loc = np.ascontiguousarray(Wf[rows, :])  # (256, 1024)
        bf_m = (
            np.ascontiguousarray(bf.reshape(8, 128).T)
            if h == 0
            else np.zeros((128, 8), np.float32)
        )

        in_maps.append(
            {
                "xT": np.ascontiguousarray(x[b].T),
                "w_qkv": w_loc,
                "b_loc": b_loc_m,
                "cosT": cos128.astype(ml_dtypes.bfloat16),
                "sinT": sin128.astype(ml_dtypes.bfloat16),
                "perm": perm.astype(ml_dtypes.bfloat16),
                "eye64": eye64.astype(ml_dtypes.bfloat16),
                "maskb": maskb.astype(ml_dtypes.bfloat16),
                "wf": wf_loc,
                "bf": bf_m,
                "onesd": np.ones((128, 64), np.float32),
            }
        )
    return in_maps


def host_unshard(results):
    """Sum the 4 per-h partial yT per batch, transpose back to (B, T, C)."""
    out = np.empty((B, T, C), np.float32)
    for b in range(B):
        acc = results[b * 4]["yT"].astype(np.float32)
        for h in range(1, 4):
            acc = acc + results[b * 4 + h]["yT"]
        out[b] = acc.T
    return out


_NC_CACHE = None


def _get_nc():
    global _NC_CACHE
    if _NC_CACHE is None:
        _NC_CACHE = build_nc()
    return _NC_CACHE


def kernel(**inputs):
    nc = _get_nc()
    in_maps = host_shard(inputs)
    res = run_bass_kernel_spmd(nc, in_maps, core_ids=list(range(8)))
    return host_unshard(res.results)
